# revision 3
# baseline (speedup 1.0000x reference)
"""4-layer GraphSAGE (mean aggr) on 8 TRN2 NeuronCores — v4.

Strategy (dst-owner node partitioning, matmul-based segmented aggregation):
  - Nodes partitioned across 8 cores by dst ownership (12500 each, padded
    to 12544 = 98*128).  fp16 node-feature tables (H=128) are replicated
    per core via AllGather each layer; layer 1 is "transform-first":
    phase A computes both table0 = x @ Wl1 (node-major) and the
    feature-major self term y0 = (x @ Wr1)^T, so every layer is uniform.
  - Per layer each core gathers the src rows of its ~200k in-edges with
    dma_gather (int16 idx -> 4 address ranges of 25088 rows).  Edges are
    sorted by (dst-block, src-range) and padded per (block, range) to
    128-edge chunks, with chunk capacities = max over cores so one SPMD
    program serves all 8 cores (pad edges gather row 0, dstrel=-1).
  - Aggregation is a matmul: per 128-edge chunk an on-chip fp16 selection
    matrix S[e, d] = (dstrel[e] == d) * invdeg[e] (2 DVE ops from small
    resident vectors) and aggT_psum[f, d] += gt[e, f].T @ S[e, d].
    PSUM (fp32) accumulates over a (block, range) group; groups drain
    into a resident fp16 SBUF accumulator agg_sb[f, 98, 128].  Mean
    normalization is folded into S via invdeg — no dma_scatter_add, no
    accumulator zeroing/readback.
  - Finalize per block: out_ps[h, d] = Wl.T @ aggT + Wr.T @ hT (self term
    streamed feature-major from DRAM), bias+ReLU on ACT, one PE transpose
    to node-major for the next table / output (fp32 at layer 4).
  - All inputs are packed into ONE fp32 blob per core (per-call dispatch
    cost ~1.6 ms per input buffer + ~0.5 ms/MB of input bytes), with
    fp16/int16 sections accessed via AP bitcast.
  - All PSUM tiles are full 2KB banks: start=True clears has_written for
    the whole bank, so concurrent accumulation groups must not share one.
"""

import numpy as np

# ---------------------------------------------------------------- constants
NCORES = 8
N = 100000
E = 1600000
F_IN = 16
H = 128
SHARD = 12500
BLK = 128
NBLK = 98                # 98*128 = 12544
SHARD_P = NBLK * BLK
TBL_ROWS = NCORES * SHARD_P   # 100352
NRANGE = 4
RANGE_ROWS = TBL_ROWS // NRANGE  # 25088 (< 2**15)
IPG = 8                  # chunks per gather instruction (1024 idxs)
HBLK = 49                # blocks per shard half
HALF_P = HBLK * BLK      # 6272
HTBL = NCORES * HALF_P   # 50176 rows per table half

_compiled = None
_plan_cache = None


# ---------------------------------------------------------------- planning
def _plan(edge_index, ipg=IPG):
    """Shared (cross-core) chunk capacities + per-core slot assignments."""
    src = np.asarray(edge_index[0], dtype=np.int64)
    dst = np.asarray(edge_index[1], dtype=np.int64)
    core = dst // SHARD
    dl = dst - core * SHARD
    blk = dl // BLK
    drel = dl - blk * BLK
    # half-major table rows: row = half*50176 + core*6272 + local, so the
    # AllGather of each shard half fills a contiguous table half (= 2 idx
    # ranges), letting AG(hi) overlap the next layer's range-0/1 gathers
    s_r = src % SHARD
    s_half = s_r // HALF_P
    srow = s_half * (NCORES * HALF_P) + (src // SHARD) * HALF_P \
        + (s_r - s_half * HALF_P)
    rng = srow // RANGE_ROWS
    sloc = srow - rng * RANGE_ROWS

    cnt = np.bincount(dst, minlength=N).astype(np.float32)
    inv_full = (1.0 / np.maximum(cnt, 1.0)).astype(np.float32)
    inv_e = inv_full[dst]

    key = (core * NBLK + blk) * NRANGE + rng
    counts = np.bincount(key, minlength=NCORES * NBLK * NRANGE).reshape(
        NCORES, NBLK, NRANGE)
    cap = -(-counts.max(axis=0) // BLK)          # [NBLK, NRANGE] chunks
    # chunk id layout: range-major, block-minor
    Qg = cap.sum(axis=0)                          # chunks per range
    range_off = np.concatenate([[0], np.cumsum(Qg)[:-1]]).astype(np.int64)
    chunk_base = np.zeros((NBLK, NRANGE), np.int64)
    for g in range(NRANGE):
        chunk_base[:, g] = range_off[g] + np.concatenate(
            [[0], np.cumsum(cap[:, g])[:-1]])
    QTOT = int(Qg.sum())

    # per-core slot arrays
    per_core = []
    for c in range(NCORES):
        m = core == c
        b_c, g_c, s_c, r_c, iv_c = blk[m], rng[m], sloc[m], drel[m], inv_e[m]
        order = np.lexsort((s_c, g_c, b_c))
        b_c, g_c, s_c, r_c, iv_c = (a[order] for a in (b_c, g_c, s_c, r_c, iv_c))
        k_bg = np.bincount(b_c * NRANGE + g_c, minlength=NBLK * NRANGE
                           ).reshape(NBLK, NRANGE)
        run_start = np.concatenate(
            [[0], np.cumsum(k_bg.reshape(-1))[:-1]]).reshape(NBLK, NRANGE)
        pos_in_run = np.arange(b_c.size) - run_start[b_c, g_c]
        slot = chunk_base[b_c, g_c] * BLK + pos_in_run
        gidx = np.zeros(QTOT * BLK, np.int16)
        drel_s = np.full(QTOT * BLK, -1.0, np.float32)
        inv_s = np.zeros(QTOT * BLK, np.float32)
        gidx[slot] = s_c.astype(np.int16)
        drel_s[slot] = r_c.astype(np.float32)
        inv_s[slot] = iv_c.astype(np.float32)
        per_core.append((gidx, drel_s, inv_s))

    # instruction grouping per range: consecutive IPG chunks
    instrs = []           # (g, q0, k, idx col offset)
    co = 0
    for g in range(NRANGE):
        q0 = int(range_off[g])
        qe = q0 + int(Qg[g])
        q = q0
        while q < qe:
            k = min(ipg, qe - q)
            instrs.append((g, q, k, co))
            co += 8 * k
            q += k
    idxcols = co
    if idxcols % 16:
        idxcols += 16 - idxcols % 16      # keep the packed view fp32-aligned

    return dict(cap=cap, chunk_base=chunk_base, Qg=Qg, QTOT=QTOT,
                range_off=range_off, instrs=instrs, idxcols=idxcols,
                per_core=per_core, ipg=ipg)


# ---------------------------------------------------------------- blob
def _ceil2(a):
    return a + (a % 2)


def _blob_layout(plan):
    """Column offsets of each section in the packed [128, cols] fp32 blob.

    Sections: xn (node-major x, fp16 [128, NBLK*16]), dr/iv (fp16
    [128, QTOT]), weights (fp16) + biases (fp32), idx (int16, packed
    [128, IDXCOLS/8]).  f16 section sizes are in fp32 columns (=2 fp16).
    """
    QTOT = plan["QTOT"]
    lay = {}
    off = 0
    lay["xn"] = off; off += NBLK * 16 // 2
    lay["dr"] = off; off += _ceil2(QTOT) // 2
    lay["iv"] = off; off += _ceil2(QTOT) // 2
    for l in range(1, 5):
        lay[f"wl{l}"] = off; off += H // 2
        lay[f"wr{l}"] = off; off += H // 2
        lay[f"b{l}"] = off; off += 1
    lay["idx"] = off; off += plan["idxcols"] // 8 // 2
    lay["cols"] = off
    return lay


# ---------------------------------------------------------------- program
def _build_program(plan, no_cc=False, gathers=True, aggs=True, finalize=True,
                   nqueues=1):
    import concourse.bacc as bacc
    import concourse.masks as masks
    import concourse.mybir as mybir
    import concourse.tile as tile

    fp32 = mybir.dt.float32
    fp16 = mybir.dt.float16
    i16 = mybir.dt.int16
    AF = mybir.ActivationFunctionType

    cap = plan["cap"]
    chunk_base = plan["chunk_base"]
    instrs = plan["instrs"]
    QTOT = plan["QTOT"]
    IDXCOLS = plan["idxcols"]
    C8 = IDXCOLS // 8
    ipg = plan["ipg"]

    nonempty = {b: [g for g in range(NRANGE) if cap[b, g] > 0]
                for b in range(NBLK)}
    meta = {}
    for b in range(NBLK):
        for g in range(NRANGE):
            cb, k = int(chunk_base[b, g]), int(cap[b, g])
            for j in range(k):
                meta[cb + j] = (b, g, j == 0, j == k - 1)

    nc = bacc.Bacc(
        "TRN2",
        target_bir_lowering=False,
        debug=False,
        enable_asserts=False,
        num_devices=NCORES,
        num_swdge_queues=nqueues,
    )

    lay = _blob_layout(plan)
    fb_d = nc.dram_tensor("fb", [128, lay["cols"]], fp32, kind="ExternalInput")
    fb = fb_d.ap()
    out_d = nc.dram_tensor("out", [SHARD_P, H], fp32, kind="ExternalOutput")

    def f16sec(off, ncols):
        return fb[:, off:off + _ceil2(ncols) // 2].bitcast(fp16)[:, :ncols]

    with tile.TileContext(nc) as tc:
        with (
            tc.tile_pool(name="dram", bufs=1, space="DRAM") as dpool,
            tc.tile_pool(name="const", bufs=1) as cpool,
        ):
            sh = [[dpool.tile([HALF_P, H], fp16, name=f"sh{l}_{h}")
                   for h in range(2)] for l in range(4)]
            tbls = [[dpool.tile([HTBL, H], fp16,
                                addr_space=("Local" if no_cc else "Shared"),
                                name=f"tbl{l}_{h}") for h in range(2)]
                    for l in range(4)]
            # feature-major self terms: ht[0] = (x @ Wr1)^T, ht[l] = h_l^T
            ht_dram = [dpool.tile([128, SHARD_P], fp16, name=f"ht{l}")
                       for l in range(4)]

            # -------- constants to SBUF
            ident = cpool.tile([128, 128], fp32)
            masks.make_identity(nc, ident[:])
            ident_h = cpool.tile([128, 128], fp16)
            nc.vector.tensor_copy(ident_h[:], ident[:])
            iota3 = cpool.tile([128, ipg, 128], fp16)
            nc.gpsimd.iota(iota3[:], pattern=[[0, ipg], [1, 128]], base=0,
                           channel_multiplier=0,
                           allow_small_or_imprecise_dtypes=True)
            idx_sb = cpool.tile([128, IDXCOLS], i16)
            idx_src = fb[:, lay["idx"]:lay["idx"] + C8 // 2].bitcast(i16)
            idx_src3 = idx_src.rearrange("(a r) c -> r a c", a=8)
            for g in range(8):
                nc.sync.dma_start(
                    idx_sb[16 * g:16 * (g + 1), :].rearrange(
                        "r (a c) -> r a c", a=8),
                    idx_src3)
            dr_sb = cpool.tile([128, QTOT], fp16)
            nc.sync.dma_start(dr_sb[:], f16sec(lay["dr"], QTOT))
            iv_sb = cpool.tile([128, QTOT], fp16)
            nc.sync.dma_start(iv_sb[:], f16sec(lay["iv"], QTOT))
            agg_sb = cpool.tile([128, NBLK, 128], fp16, name="agg")
            xn_sb = cpool.tile([128, NBLK * 16], fp16, name="xn")
            nc.sync.dma_start(xn_sb[:], f16sec(lay["xn"], NBLK * 16))
            xnv = xn_sb.rearrange("p (b f) -> p b f", b=NBLK)
            w_sb = {}
            for l in range(1, 5):
                din = F_IN if l == 1 else H
                for nm in (f"wl{l}", f"wr{l}"):
                    t = cpool.tile([din, H], fp16, name=f"{nm}_sb")
                    nc.sync.dma_start(t[:], f16sec(lay[nm], H)[0:din, :])
                    w_sb[nm] = t
                t = cpool.tile([128, 1], fp32, name=f"b{l}_sb")
                nc.sync.dma_start(t[:], fb[:, lay[f"b{l}"]:lay[f"b{l}"] + 1])
                w_sb[f"b{l}"] = t

            def nm_view(dram_ap):
                return dram_ap.rearrange("(b p) f -> p b f", p=128)

            def allgather(src, dst):
                if no_cc:
                    nc.sync.dma_start(dst[:HALF_P, :], src[:, :])
                    return
                nc.gpsimd.collective_compute(
                    "AllGather",
                    mybir.AluOpType.bypass,
                    replica_groups=[list(range(NCORES))],
                    ins=[src.opt()],
                    outs=[dst.opt()],
                )

            # ---- phase A: table0 = x @ Wl1 (node-major) and
            #               y0^T = (x @ Wr1)^T (feature-major) from
            #               node-major fp16 x resident in SBUF
            with (
                tc.tile_pool(name="phA", bufs=3) as apool,
                tc.tile_pool(name="psA", bufs=2, space="PSUM") as pApool,
                tc.tile_pool(name="psY", bufs=2, space="PSUM") as pYpool,
                tc.tile_pool(name="ptA", bufs=1, space="PSUM") as ptApool,
            ):
                sh0v = [nm_view(s) for s in sh[0]]
                for b0 in range(0, NBLK, 4):
                    nb = min(4, NBLK - b0)
                    cw = nb * 128
                    psA = pApool.tile([128, 512], fp32, tag="psA")
                    psY = pYpool.tile([128, 512], fp32, tag="psY")
                    for j in range(nb):
                        b = b0 + j
                        ptx = ptApool.tile([128, 1024], fp16, tag="ptx")
                        nc.tensor.transpose(ptx[0:16, 0:128], xnv[:, b, :],
                                            ident_h[:])
                        xT = apool.tile([16, 128], fp16, tag="xT")
                        nc.vector.tensor_copy(xT[:], ptx[0:16, 0:128])
                        # disjoint column writes into one bank: only the
                        # first matmul clears has_written
                        nc.tensor.matmul(psA[:, j * 128:(j + 1) * 128],
                                         w_sb["wl1"][:], xT[:],
                                         start=(j == 0), stop=(j == nb - 1))
                        nc.tensor.matmul(psY[:, j * 128:(j + 1) * 128],
                                         w_sb["wr1"][:], xT[:],
                                         start=(j == 0), stop=(j == nb - 1))
                    yt = apool.tile([128, 512], fp16, tag="yt")
                    nc.scalar.copy(yt[:, :cw], psY[:, :cw])
                    nc.sync.dma_start(
                        ht_dram[0][:, b0 * 128:b0 * 128 + cw], yt[:, :cw])
                    tmp = apool.tile([128, 512], fp16, tag="x1tmp")
                    nc.scalar.copy(tmp[:, :cw], psA[:, :cw])
                    ptt = ptApool.tile([128, 1024], fp16, tag="ptt")
                    for j in range(nb):
                        nc.tensor.transpose(
                            ptt[:, j * 128:(j + 1) * 128],
                            tmp[:, j * 128:(j + 1) * 128], ident_h[:])
                    stage = apool.tile([128, 4, 128], fp16, tag="stA")
                    nc.vector.tensor_copy(
                        stage[:, :nb, :],
                        ptt.rearrange("p (a b) -> p a b", a=8)[:, :nb, :])
                    for j in range(nb):
                        b = b0 + j
                        nc.sync.dma_start(
                            sh0v[b // HBLK][:, b % HBLK, :],
                            stage[:, j, :])
                allgather(sh[0][0], tbls[0][0])
                allgather(sh[0][1], tbls[0][1])

            aggv = agg_sb  # [128, NBLK, 128] fp16

            # ---- layers
            with (
                tc.tile_pool(name="gat", bufs=6) as gpool,
                tc.tile_pool(name="sel", bufs=4) as spool,
                tc.tile_pool(name="work", bufs=3) as wpool,
                tc.tile_pool(name="psum_a", bufs=4, space="PSUM") as ppool,
                tc.tile_pool(name="psum_o", bufs=2, space="PSUM") as popool,
                tc.tile_pool(name="psum_t", bufs=1, space="PSUM") as ptpool,
            ):
                for l in range(1, 5):
                    table = tbls[l - 1]
                    for b in range(NBLK):
                        if not nonempty[b]:
                            nc.vector.memset(aggv[:, b, :], 0.0)

                    # phase B
                    open_ps = {}
                    for qi, (g, q0, k, co) in enumerate(instrs):
                        gh = g - 2 * (g // 2)
                        tslice = table[g // 2][
                            gh * RANGE_ROWS:(gh + 1) * RANGE_ROWS, :]
                        gt = gpool.tile([128, ipg, 128], fp16, tag="gt")
                        if gathers:
                            nc.gpsimd.dma_gather(
                                gt[:, :k, :], tslice, idx_sb[:, co:co + 8 * k],
                                num_idxs=128 * k, num_idxs_reg=128 * k,
                                elem_size=H, queue_num=qi % nqueues,
                            )
                        else:
                            nc.vector.memset(gt[:, :k, :], 0.0)
                        if not aggs:
                            continue
                        sm = spool.tile([128, ipg, 128], fp16, tag="sm")
                        nc.vector.tensor_tensor(
                            sm[:, :k, :],
                            dr_sb[:, q0:q0 + k].to_broadcast([128, k, 128]),
                            iota3[:, :k, :],
                            mybir.AluOpType.is_equal)
                        nc.vector.tensor_mul(
                            sm[:, :k, :], sm[:, :k, :],
                            iv_sb[:, q0:q0 + k].to_broadcast([128, k, 128]))
                        for j in range(k):
                            q = q0 + j
                            b, g_, first, last = meta[q]
                            if first:
                                psnew = ppool.tile([128, 512], fp32, tag="pag")
                                open_ps[(b, g_)] = psnew
                            ps = open_ps[(b, g_)]
                            nc.tensor.matmul(ps[:, :128], gt[:, j, :],
                                             sm[:, j, :],
                                             start=first, stop=last)
                            if last:
                                del open_ps[(b, g_)]
                                if g_ == nonempty[b][0]:
                                    nc.vector.tensor_copy(
                                        aggv[:, b, :], ps[:, :128])
                                else:
                                    nc.vector.tensor_add(
                                        aggv[:, b, :], aggv[:, b, :],
                                        ps[:, :128])

                    # finalize per block
                    if l < 4:
                        dst_views = [nm_view(s) for s in sh[l]]
                    else:
                        ov = nm_view(out_d.ap())
                        dst_views = [ov[:, :HBLK, :], ov[:, HBLK:, :]]
                    func = AF.Relu if l < 4 else AF.Identity
                    wl_t = ident_h if l == 1 else w_sb[f"wl{l}"]
                    wr_t = ident_h if l == 1 else w_sb[f"wr{l}"]
                    for b in range(NBLK) if finalize else []:
                        hc = wpool.tile([128, 128], fp16, tag="hc")
                        nc.sync.dma_start(
                            hc[:], ht_dram[l - 1][:, b * 128:(b + 1) * 128])
                        ps = popool.tile([128, 512], fp32, tag="po")
                        nc.tensor.matmul(ps[:, :128], wl_t[:], aggv[:, b, :],
                                         start=True, stop=False)
                        nc.tensor.matmul(ps[:, :128], wr_t[:], hc[:],
                                         start=False, stop=True)
                        if l < 4:
                            ev = wpool.tile([128, 128], fp16, tag="ev")
                            nc.scalar.activation(ev[:], ps[:, :128], func,
                                                 bias=w_sb[f"b{l}"][:])
                            nc.sync.dma_start(
                                ht_dram[l][:, b * 128:(b + 1) * 128], ev[:])
                            pt = ptpool.tile([128, 1024], fp16, tag="ptf")
                            nc.tensor.transpose(pt[:, :128], ev[:], ident_h[:])
                            stage = wpool.tile([128, 128], fp16, tag="stg")
                            nc.vector.tensor_copy(stage[:], pt[:, :128])
                            nc.sync.dma_start(
                                dst_views[b // HBLK][:, b % HBLK, :], stage[:])
                            if b == HBLK - 1:
                                allgather(sh[l][0], tbls[l][0])
                        else:
                            ev = wpool.tile([128, 128], fp32, tag="ev4")
                            nc.scalar.activation(ev[:], ps[:, :128], func,
                                                 bias=w_sb[f"b{l}"][:])
                            pt = ptpool.tile([128, 512], fp32, tag="ptf4")
                            nc.tensor.transpose(pt[:, :128], ev[:], ident[:])
                            stage = wpool.tile([128, 128], fp32, tag="stg4")
                            nc.vector.tensor_copy(stage[:], pt[:, :128])
                            nc.sync.dma_start(
                                dst_views[b // HBLK][:, b % HBLK, :], stage[:])
                    if l < 4:
                        allgather(sh[l][1], tbls[l][1])

    nc.compile()
    return nc


# ---------------------------------------------------------------- host side
def make_in_maps(x, edge_index, weights, plan):
    x = np.asarray(x, dtype=np.float32)
    instrs = plan["instrs"]
    QTOT = plan["QTOT"]
    IDXCOLS = plan["idxcols"]
    C8 = IDXCOLS // 8
    ipg = plan["ipg"]
    lay = _blob_layout(plan)

    def put16(fbm, off, arr):
        """fp16 array [rows, cols] -> fp32 view at column offset."""
        a = np.ascontiguousarray(arr, np.float16)
        if a.shape[1] % 2:
            a = np.concatenate(
                [a, np.zeros((a.shape[0], 1), np.float16)], axis=1)
        v = np.ascontiguousarray(a).view(np.float32)
        fbm[0:a.shape[0], off:off + v.shape[1]] = v

    in_maps = []
    for c in range(NCORES):
        gidx, drel_s, inv_s = plan["per_core"][c]
        cols = []
        for (g, q0, k, co) in instrs:
            a = gidx[q0 * BLK:(q0 + k) * BLK]
            cols.append(a.reshape(-1, 16).T)
        idx_w = np.concatenate(cols, axis=1)       # [16, <=IDXCOLS]
        if idx_w.shape[1] < IDXCOLS:
            idx_w = np.concatenate(
                [idx_w, np.zeros((16, IDXCOLS - idx_w.shape[1]), np.int16)],
                axis=1)
        # pack [16, IDXCOLS] -> [128, C8]: row 16a+r = idx_w[r, a*C8:(a+1)*C8]
        pk = np.ascontiguousarray(
            idx_w.reshape(16, 8, C8).transpose(1, 0, 2).reshape(128, C8))

        fbm = np.zeros((128, lay["cols"]), np.float32)
        # node-major x: [12544, 16] -> [98, 128, 16] -> [128, 98*16]
        xs = np.zeros((SHARD_P, F_IN), np.float32)
        xs[:SHARD] = x[c * SHARD:(c + 1) * SHARD]
        xn = xs.reshape(NBLK, 128, F_IN).transpose(1, 0, 2).reshape(128, -1)
        put16(fbm, lay["xn"], xn)
        put16(fbm, lay["dr"], drel_s.reshape(QTOT, BLK).T)
        put16(fbm, lay["iv"], inv_s.reshape(QTOT, BLK).T)
        for l in range(1, 5):
            put16(fbm, lay[f"wl{l}"], np.asarray(weights[f"Wl{l}"]))
            put16(fbm, lay[f"wr{l}"], np.asarray(weights[f"Wr{l}"]))
            fbm[:, lay[f"b{l}"]] = np.asarray(
                weights[f"b{l}"], np.float32).reshape(128)
        fbm[:, lay["idx"]:lay["idx"] + C8 // 2] = pk.view(np.float32)
        in_maps.append({"fb": fbm})
    return in_maps


def get_program_and_maps(x, edge_index, weights):
    global _compiled, _plan_cache
    if _plan_cache is None:
        _plan_cache = _plan(edge_index)
    if _compiled is None:
        _compiled = _build_program(_plan_cache, nqueues=4)
    return _compiled, make_in_maps(x, edge_index, weights, _plan_cache)


def kernel(x, edge_index, Wl1, Wr1, b1, Wl2, Wr2, b2, Wl3, Wr3, b3,
           Wl4, Wr4, b4, _trace=False, _trace_kwargs=None):
    from concourse.bass_utils import run_bass_kernel_spmd

    weights = {
        "Wl1": Wl1, "Wr1": Wr1, "b1": b1,
        "Wl2": Wl2, "Wr2": Wr2, "b2": b2,
        "Wl3": Wl3, "Wr3": Wr3, "b3": b3,
        "Wl4": Wl4, "Wr4": Wr4, "b4": b4,
    }
    nc, in_maps = get_program_and_maps(x, edge_index, weights)
    res = run_bass_kernel_spmd(
        nc, in_maps, core_ids=list(range(NCORES)),
        trace=_trace, **(_trace_kwargs or {}),
    )
    shards = [res.results[c]["out"][:SHARD] for c in range(NCORES)]
    out = np.concatenate(shards, axis=0).astype(np.float32)
    if _trace:
        return out, res
    return out


# revision 4
# speedup vs baseline: 1.0832x; 1.0832x over previous
"""4-layer GraphSAGE (mean aggr) on 8 TRN2 NeuronCores — v4.

Strategy (dst-owner node partitioning, matmul-based segmented aggregation):
  - Nodes partitioned across 8 cores by dst ownership (12500 each, padded
    to 12544 = 98*128).  fp16 node-feature tables (H=128) are replicated
    per core via AllGather each layer; layer 1 is "transform-first":
    phase A computes both table0 = x @ Wl1 (node-major) and the
    feature-major self term y0 = (x @ Wr1)^T, so every layer is uniform.
  - Per layer each core gathers the src rows of its ~200k in-edges with
    dma_gather (int16 idx -> 4 address ranges of 25088 rows).  Edges are
    sorted by (dst-block, src-range) and padded per (block, range) to
    128-edge chunks, with chunk capacities = max over cores so one SPMD
    program serves all 8 cores (pad edges gather row 0, dstrel=-1).
  - Aggregation is a matmul: per 128-edge chunk an on-chip fp16 selection
    matrix S[e, d] = (dstrel[e] == d) * invdeg[e] (2 DVE ops from small
    resident vectors) and aggT_psum[f, d] += gt[e, f].T @ S[e, d].
    PSUM (fp32) accumulates over a (block, range) group; groups drain
    into a resident fp16 SBUF accumulator agg_sb[f, 98, 128].  Mean
    normalization is folded into S via invdeg — no dma_scatter_add, no
    accumulator zeroing/readback.
  - Finalize per block: out_ps[h, d] = Wl.T @ aggT + Wr.T @ hT (self term
    streamed feature-major from DRAM), bias+ReLU on ACT, one PE transpose
    to node-major for the next table / output (fp32 at layer 4).
  - All inputs are packed into ONE fp32 blob per core (per-call dispatch
    cost ~1.6 ms per input buffer + ~0.5 ms/MB of input bytes), with
    fp16/int16 sections accessed via AP bitcast.
  - All PSUM tiles are full 2KB banks: start=True clears has_written for
    the whole bank, so concurrent accumulation groups must not share one.
"""

import numpy as np

# ---------------------------------------------------------------- constants
NCORES = 8
N = 100000
E = 1600000
F_IN = 16
H = 128
SHARD = 12500
BLK = 128
NBLK = 98                # 98*128 = 12544
SHARD_P = NBLK * BLK
TBL_ROWS = NCORES * SHARD_P   # 100352
NRANGE = 4
RANGE_ROWS = TBL_ROWS // NRANGE  # 25088 (< 2**15)
IPG = 8                  # chunks per gather instruction (1024 idxs)
HBLK = 49                # blocks per shard half
HALF_P = HBLK * BLK      # 6272
HTBL = NCORES * HALF_P   # 50176 rows per table half

_compiled = None
_plan_cache = None


# ---------------------------------------------------------------- planning
def _plan(edge_index, ipg=IPG):
    """Shared (cross-core) chunk capacities + per-core slot assignments."""
    src = np.asarray(edge_index[0], dtype=np.int64)
    dst = np.asarray(edge_index[1], dtype=np.int64)
    core = dst // SHARD
    dl = dst - core * SHARD
    blk = dl // BLK
    drel = dl - blk * BLK
    # half-major table rows: row = half*50176 + core*6272 + local, so the
    # AllGather of each shard half fills a contiguous table half (= 2 idx
    # ranges), letting AG(hi) overlap the next layer's range-0/1 gathers
    s_r = src % SHARD
    s_half = s_r // HALF_P
    srow = s_half * (NCORES * HALF_P) + (src // SHARD) * HALF_P \
        + (s_r - s_half * HALF_P)
    rng = srow // RANGE_ROWS
    sloc = srow - rng * RANGE_ROWS

    cnt = np.bincount(dst, minlength=N).astype(np.float32)
    inv_full = (1.0 / np.maximum(cnt, 1.0)).astype(np.float32)
    inv_e = inv_full[dst]

    key = (core * NBLK + blk) * NRANGE + rng
    counts = np.bincount(key, minlength=NCORES * NBLK * NRANGE).reshape(
        NCORES, NBLK, NRANGE)
    cap = -(-counts.max(axis=0) // BLK)          # [NBLK, NRANGE] chunks
    # chunk id layout: range-major, block-minor
    Qg = cap.sum(axis=0)                          # chunks per range
    range_off = np.concatenate([[0], np.cumsum(Qg)[:-1]]).astype(np.int64)
    chunk_base = np.zeros((NBLK, NRANGE), np.int64)
    for g in range(NRANGE):
        chunk_base[:, g] = range_off[g] + np.concatenate(
            [[0], np.cumsum(cap[:, g])[:-1]])
    QTOT = int(Qg.sum())

    # per-core slot arrays
    per_core = []
    for c in range(NCORES):
        m = core == c
        b_c, g_c, s_c, r_c, iv_c = blk[m], rng[m], sloc[m], drel[m], inv_e[m]
        order = np.lexsort((s_c, g_c, b_c))
        b_c, g_c, s_c, r_c, iv_c = (a[order] for a in (b_c, g_c, s_c, r_c, iv_c))
        k_bg = np.bincount(b_c * NRANGE + g_c, minlength=NBLK * NRANGE
                           ).reshape(NBLK, NRANGE)
        run_start = np.concatenate(
            [[0], np.cumsum(k_bg.reshape(-1))[:-1]]).reshape(NBLK, NRANGE)
        pos_in_run = np.arange(b_c.size) - run_start[b_c, g_c]
        slot = chunk_base[b_c, g_c] * BLK + pos_in_run
        gidx = np.zeros(QTOT * BLK, np.int16)
        drel_s = np.full(QTOT * BLK, -1.0, np.float32)
        inv_s = np.zeros(QTOT * BLK, np.float32)
        gidx[slot] = s_c.astype(np.int16)
        drel_s[slot] = r_c.astype(np.float32)
        inv_s[slot] = iv_c.astype(np.float32)
        per_core.append((gidx, drel_s, inv_s))

    # instruction grouping per range: consecutive IPG chunks
    instrs = []           # (g, q0, k, idx col offset)
    co = 0
    for g in range(NRANGE):
        q0 = int(range_off[g])
        qe = q0 + int(Qg[g])
        q = q0
        while q < qe:
            k = min(ipg, qe - q)
            instrs.append((g, q, k, co))
            co += 8 * k
            q += k
    idxcols = co
    if idxcols % 16:
        idxcols += 16 - idxcols % 16      # keep the packed view fp32-aligned

    return dict(cap=cap, chunk_base=chunk_base, Qg=Qg, QTOT=QTOT,
                range_off=range_off, instrs=instrs, idxcols=idxcols,
                per_core=per_core, ipg=ipg)


# ---------------------------------------------------------------- blob
def _ceil2(a):
    return a + (a % 2)


def _blob_layout(plan):
    """Column offsets of each section in the packed [128, cols] fp32 blob.

    Sections: xn (node-major x, fp16 [128, NBLK*16]), dr/iv (fp16
    [128, QTOT]), weights (fp16) + biases (fp32), idx (int16, packed
    [128, IDXCOLS/8]).  f16 section sizes are in fp32 columns (=2 fp16).
    """
    QTOT = plan["QTOT"]
    lay = {}
    off = 0
    lay["xn"] = off; off += NBLK * 16 // 2
    lay["dr"] = off; off += _ceil2(QTOT) // 2
    lay["iv"] = off; off += _ceil2(QTOT) // 2
    for l in range(1, 5):
        lay[f"wl{l}"] = off; off += H // 2
        lay[f"wr{l}"] = off; off += H // 2
        lay[f"b{l}"] = off; off += 1
    lay["idx"] = off; off += plan["idxcols"] // 8 // 2
    lay["cols"] = off
    return lay


# ---------------------------------------------------------------- program
def _build_program(plan, no_cc=False, gathers=True, aggs=True, finalize=True,
                   nqueues=1):
    import concourse.bacc as bacc
    import concourse.masks as masks
    import concourse.mybir as mybir
    import concourse.tile as tile

    fp32 = mybir.dt.float32
    fp16 = mybir.dt.float16
    i16 = mybir.dt.int16
    AF = mybir.ActivationFunctionType

    cap = plan["cap"]
    chunk_base = plan["chunk_base"]
    instrs = plan["instrs"]
    QTOT = plan["QTOT"]
    IDXCOLS = plan["idxcols"]
    C8 = IDXCOLS // 8
    ipg = plan["ipg"]

    nonempty = {b: [g for g in range(NRANGE) if cap[b, g] > 0]
                for b in range(NBLK)}
    meta = {}
    for b in range(NBLK):
        for g in range(NRANGE):
            cb, k = int(chunk_base[b, g]), int(cap[b, g])
            for j in range(k):
                meta[cb + j] = (b, g, j == 0, j == k - 1)

    nc = bacc.Bacc(
        "TRN2",
        target_bir_lowering=False,
        debug=False,
        enable_asserts=False,
        num_devices=NCORES,
        num_swdge_queues=nqueues,
    )

    lay = _blob_layout(plan)
    fb_d = nc.dram_tensor("fb", [128, lay["cols"]], fp32, kind="ExternalInput")
    fb = fb_d.ap()
    out_d = nc.dram_tensor("out", [SHARD_P, H], fp32, kind="ExternalOutput")

    def f16sec(off, ncols):
        return fb[:, off:off + _ceil2(ncols) // 2].bitcast(fp16)[:, :ncols]

    with tile.TileContext(nc) as tc:
        with (
            tc.tile_pool(name="dram", bufs=1, space="DRAM") as dpool,
            tc.tile_pool(name="const", bufs=1) as cpool,
        ):
            sh = [[dpool.tile([HALF_P, H], fp16, name=f"sh{l}_{h}")
                   for h in range(2)] for l in range(4)]
            tbls = [[dpool.tile([HTBL, H], fp16,
                                addr_space=("Local" if no_cc else "Shared"),
                                name=f"tbl{l}_{h}") for h in range(2)]
                    for l in range(4)]
            # feature-major self terms: ht[0] = (x @ Wr1)^T, ht[l] = h_l^T
            ht_dram = [dpool.tile([128, SHARD_P], fp16, name=f"ht{l}")
                       for l in range(4)]

            # -------- constants to SBUF
            ident = cpool.tile([128, 128], fp32)
            masks.make_identity(nc, ident[:])
            ident_h = cpool.tile([128, 128], fp16)
            nc.vector.tensor_copy(ident_h[:], ident[:])
            iota3 = cpool.tile([128, ipg, 128], fp16)
            nc.gpsimd.iota(iota3[:], pattern=[[0, ipg], [1, 128]], base=0,
                           channel_multiplier=0,
                           allow_small_or_imprecise_dtypes=True)
            idx_sb = cpool.tile([128, IDXCOLS], i16)
            idx_src = fb[:, lay["idx"]:lay["idx"] + C8 // 2].bitcast(i16)
            idx_src3 = idx_src.rearrange("(a r) c -> r a c", a=8)
            for g in range(8):
                nc.sync.dma_start(
                    idx_sb[16 * g:16 * (g + 1), :].rearrange(
                        "r (a c) -> r a c", a=8),
                    idx_src3)
            dr_sb = cpool.tile([128, QTOT], fp16)
            nc.sync.dma_start(dr_sb[:], f16sec(lay["dr"], QTOT))
            iv_sb = cpool.tile([128, QTOT], fp16)
            nc.sync.dma_start(iv_sb[:], f16sec(lay["iv"], QTOT))
            agg_sb = cpool.tile([128, NBLK, 128], fp16, name="agg")
            xn_sb = cpool.tile([128, NBLK * 16], fp16, name="xn")
            nc.sync.dma_start(xn_sb[:], f16sec(lay["xn"], NBLK * 16))
            xnv = xn_sb.rearrange("p (b f) -> p b f", b=NBLK)
            w_sb = {}
            for l in range(1, 5):
                din = F_IN if l == 1 else H
                for nm in (f"wl{l}", f"wr{l}"):
                    t = cpool.tile([din, H], fp16, name=f"{nm}_sb")
                    nc.sync.dma_start(t[:], f16sec(lay[nm], H)[0:din, :])
                    w_sb[nm] = t
                t = cpool.tile([128, 1], fp32, name=f"b{l}_sb")
                nc.sync.dma_start(t[:], fb[:, lay[f"b{l}"]:lay[f"b{l}"] + 1])
                w_sb[f"b{l}"] = t

            def nm_view(dram_ap):
                return dram_ap.rearrange("(b p) f -> p b f", p=128)

            def allgather(src, dst):
                if no_cc:
                    nc.sync.dma_start(dst[:HALF_P, :], src[:, :])
                    return
                nc.gpsimd.collective_compute(
                    "AllGather",
                    mybir.AluOpType.bypass,
                    replica_groups=[list(range(NCORES))],
                    ins=[src.opt()],
                    outs=[dst.opt()],
                )

            # ---- phase A: table0 = x @ Wl1 (node-major) and
            #               y0^T = (x @ Wr1)^T (feature-major) from
            #               node-major fp16 x resident in SBUF
            with (
                tc.tile_pool(name="phA", bufs=3) as apool,
                tc.tile_pool(name="psA", bufs=2, space="PSUM") as pApool,
                tc.tile_pool(name="psY", bufs=2, space="PSUM") as pYpool,
                tc.tile_pool(name="ptA", bufs=1, space="PSUM") as ptApool,
            ):
                sh0v = [nm_view(s) for s in sh[0]]
                for b0 in range(0, NBLK, 4):
                    nb = min(4, NBLK - b0)
                    cw = nb * 128
                    psA = pApool.tile([128, 512], fp32, tag="psA")
                    psY = pYpool.tile([128, 512], fp32, tag="psY")
                    for j in range(nb):
                        b = b0 + j
                        ptx = ptApool.tile([128, 1024], fp16, tag="ptx")
                        nc.tensor.transpose(ptx[0:16, 0:128], xnv[:, b, :],
                                            ident_h[:])
                        xT = apool.tile([16, 128], fp16, tag="xT")
                        nc.vector.tensor_copy(xT[:], ptx[0:16, 0:128])
                        # disjoint column writes into one bank: only the
                        # first matmul clears has_written
                        nc.tensor.matmul(psA[:, j * 128:(j + 1) * 128],
                                         w_sb["wl1"][:], xT[:],
                                         start=(j == 0), stop=(j == nb - 1))
                        nc.tensor.matmul(psY[:, j * 128:(j + 1) * 128],
                                         w_sb["wr1"][:], xT[:],
                                         start=(j == 0), stop=(j == nb - 1))
                    yt = apool.tile([128, 512], fp16, tag="yt")
                    nc.scalar.copy(yt[:, :cw], psY[:, :cw])
                    nc.sync.dma_start(
                        ht_dram[0][:, b0 * 128:b0 * 128 + cw], yt[:, :cw])
                    tmp = apool.tile([128, 512], fp16, tag="x1tmp")
                    nc.scalar.copy(tmp[:, :cw], psA[:, :cw])
                    ptt = ptApool.tile([128, 1024], fp16, tag="ptt")
                    for j in range(nb):
                        nc.tensor.transpose(
                            ptt[:, j * 128:(j + 1) * 128],
                            tmp[:, j * 128:(j + 1) * 128], ident_h[:])
                    stage = apool.tile([128, 4, 128], fp16, tag="stA")
                    nc.vector.tensor_copy(
                        stage[:, :nb, :],
                        ptt.rearrange("p (a b) -> p a b", a=8)[:, :nb, :])
                    for j in range(nb):
                        b = b0 + j
                        nc.sync.dma_start(
                            sh0v[b // HBLK][:, b % HBLK, :],
                            stage[:, j, :])
                allgather(sh[0][0], tbls[0][0])
                allgather(sh[0][1], tbls[0][1])

            aggv = agg_sb  # [128, NBLK, 128] fp16

            # ---- layers
            with (
                tc.tile_pool(name="gat", bufs=6) as gpool,
                tc.tile_pool(name="sel", bufs=4) as spool,
                tc.tile_pool(name="work", bufs=3) as wpool,
                tc.tile_pool(name="psum_a", bufs=4, space="PSUM") as ppool,
                tc.tile_pool(name="psum_o", bufs=2, space="PSUM") as popool,
                tc.tile_pool(name="psum_t", bufs=1, space="PSUM") as ptpool,
            ):
                for l in range(1, 5):
                    table = tbls[l - 1]
                    for b in range(NBLK):
                        if not nonempty[b]:
                            nc.vector.memset(aggv[:, b, :], 0.0)

                    if l < 4:
                        dst_views = [nm_view(s) for s in sh[l]]
                    else:
                        ov = nm_view(out_d.ap())
                        dst_views = [ov[:, :HBLK, :], ov[:, HBLK:, :]]
                    func = AF.Relu if l < 4 else AF.Identity
                    wl_t = ident_h if l == 1 else w_sb[f"wl{l}"]
                    wr_t = ident_h if l == 1 else w_sb[f"wr{l}"]

                    def emit_finalize(b, l=l, dst_views=dst_views, func=func,
                                      wl_t=wl_t, wr_t=wr_t):
                        hc = wpool.tile([128, 128], fp16, tag="hc")
                        nc.sync.dma_start(
                            hc[:], ht_dram[l - 1][:, b * 128:(b + 1) * 128])
                        ps = popool.tile([128, 512], fp32, tag="po")
                        nc.tensor.matmul(ps[:, :128], wl_t[:], aggv[:, b, :],
                                         start=True, stop=False)
                        nc.tensor.matmul(ps[:, :128], wr_t[:], hc[:],
                                         start=False, stop=True)
                        if l < 4:
                            ev = wpool.tile([128, 128], fp16, tag="ev")
                            nc.scalar.activation(ev[:], ps[:, :128], func,
                                                 bias=w_sb[f"b{l}"][:])
                            nc.sync.dma_start(
                                ht_dram[l][:, b * 128:(b + 1) * 128], ev[:])
                            pt = ptpool.tile([128, 1024], fp16, tag="ptf")
                            nc.tensor.transpose(pt[:, :128], ev[:], ident_h[:])
                            stage = wpool.tile([128, 128], fp16, tag="stg")
                            nc.vector.tensor_copy(stage[:], pt[:, :128])
                            nc.sync.dma_start(
                                dst_views[b // HBLK][:, b % HBLK, :], stage[:])
                        else:
                            ev = wpool.tile([128, 128], fp32, tag="ev4")
                            nc.scalar.activation(ev[:], ps[:, :128], func,
                                                 bias=w_sb[f"b{l}"][:])
                            pt = ptpool.tile([128, 512], fp32, tag="ptf4")
                            nc.tensor.transpose(pt[:, :128], ev[:], ident[:])
                            stage = wpool.tile([128, 128], fp32, tag="stg4")
                            nc.vector.tensor_copy(stage[:], pt[:, :128])
                            nc.sync.dma_start(
                                dst_views[b // HBLK][:, b % HBLK, :], stage[:])

                    # phase B, with finalize interleaved: each block finalizes
                    # as soon as its last range-group drains, so sh_lo is
                    # complete mid-pass and AG(lo) launches early
                    pend_lo = set(range(HBLK))
                    pend_hi = set(range(HBLK, NBLK))
                    if finalize:
                        # blocks with no edges anywhere: agg is memset above
                        for b in range(NBLK):
                            if not nonempty[b]:
                                emit_finalize(b)
                                (pend_lo if b < HBLK else pend_hi).discard(b)
                    open_ps = {}
                    for qi, (g, q0, k, co) in enumerate(instrs):
                        gh = g - 2 * (g // 2)
                        tslice = table[g // 2][
                            gh * RANGE_ROWS:(gh + 1) * RANGE_ROWS, :]
                        gt = gpool.tile([128, ipg, 128], fp16, tag="gt")
                        if gathers:
                            nc.gpsimd.dma_gather(
                                gt[:, :k, :], tslice, idx_sb[:, co:co + 8 * k],
                                num_idxs=128 * k, num_idxs_reg=128 * k,
                                elem_size=H, queue_num=qi % nqueues,
                            )
                        else:
                            nc.vector.memset(gt[:, :k, :], 0.0)
                        if not aggs:
                            continue
                        sm = spool.tile([128, ipg, 128], fp16, tag="sm")
                        nc.vector.tensor_tensor(
                            sm[:, :k, :],
                            dr_sb[:, q0:q0 + k].to_broadcast([128, k, 128]),
                            iota3[:, :k, :],
                            mybir.AluOpType.is_equal)
                        nc.vector.tensor_mul(
                            sm[:, :k, :], sm[:, :k, :],
                            iv_sb[:, q0:q0 + k].to_broadcast([128, k, 128]))
                        for j in range(k):
                            q = q0 + j
                            b, g_, first, last = meta[q]
                            if first:
                                psnew = ppool.tile([128, 512], fp32, tag="pag")
                                open_ps[(b, g_)] = psnew
                            ps = open_ps[(b, g_)]
                            nc.tensor.matmul(ps[:, :128], gt[:, j, :],
                                             sm[:, j, :],
                                             start=first, stop=last)
                            if last:
                                del open_ps[(b, g_)]
                                if g_ == nonempty[b][0]:
                                    nc.vector.tensor_copy(
                                        aggv[:, b, :], ps[:, :128])
                                else:
                                    nc.vector.tensor_add(
                                        aggv[:, b, :], aggv[:, b, :],
                                        ps[:, :128])
                                if finalize and g_ == nonempty[b][-1]:
                                    emit_finalize(b)
                                    pend = pend_lo if b < HBLK else pend_hi
                                    pend.discard(b)
                                    if l < 4 and not pend_lo and b < HBLK:
                                        allgather(sh[l][0], tbls[l][0])
                    if finalize and l < 4 and pend_lo:
                        allgather(sh[l][0], tbls[l][0])
                    if l < 4:
                        allgather(sh[l][1], tbls[l][1])

    nc.compile()
    return nc


# ---------------------------------------------------------------- host side
def make_in_maps(x, edge_index, weights, plan):
    x = np.asarray(x, dtype=np.float32)
    instrs = plan["instrs"]
    QTOT = plan["QTOT"]
    IDXCOLS = plan["idxcols"]
    C8 = IDXCOLS // 8
    ipg = plan["ipg"]
    lay = _blob_layout(plan)

    def put16(fbm, off, arr):
        """fp16 array [rows, cols] -> fp32 view at column offset."""
        a = np.ascontiguousarray(arr, np.float16)
        if a.shape[1] % 2:
            a = np.concatenate(
                [a, np.zeros((a.shape[0], 1), np.float16)], axis=1)
        v = np.ascontiguousarray(a).view(np.float32)
        fbm[0:a.shape[0], off:off + v.shape[1]] = v

    in_maps = []
    for c in range(NCORES):
        gidx, drel_s, inv_s = plan["per_core"][c]
        cols = []
        for (g, q0, k, co) in instrs:
            a = gidx[q0 * BLK:(q0 + k) * BLK]
            cols.append(a.reshape(-1, 16).T)
        idx_w = np.concatenate(cols, axis=1)       # [16, <=IDXCOLS]
        if idx_w.shape[1] < IDXCOLS:
            idx_w = np.concatenate(
                [idx_w, np.zeros((16, IDXCOLS - idx_w.shape[1]), np.int16)],
                axis=1)
        # pack [16, IDXCOLS] -> [128, C8]: row 16a+r = idx_w[r, a*C8:(a+1)*C8]
        pk = np.ascontiguousarray(
            idx_w.reshape(16, 8, C8).transpose(1, 0, 2).reshape(128, C8))

        fbm = np.zeros((128, lay["cols"]), np.float32)
        # node-major x: [12544, 16] -> [98, 128, 16] -> [128, 98*16]
        xs = np.zeros((SHARD_P, F_IN), np.float32)
        xs[:SHARD] = x[c * SHARD:(c + 1) * SHARD]
        xn = xs.reshape(NBLK, 128, F_IN).transpose(1, 0, 2).reshape(128, -1)
        put16(fbm, lay["xn"], xn)
        put16(fbm, lay["dr"], drel_s.reshape(QTOT, BLK).T)
        put16(fbm, lay["iv"], inv_s.reshape(QTOT, BLK).T)
        for l in range(1, 5):
            put16(fbm, lay[f"wl{l}"], np.asarray(weights[f"Wl{l}"]))
            put16(fbm, lay[f"wr{l}"], np.asarray(weights[f"Wr{l}"]))
            fbm[:, lay[f"b{l}"]] = np.asarray(
                weights[f"b{l}"], np.float32).reshape(128)
        fbm[:, lay["idx"]:lay["idx"] + C8 // 2] = pk.view(np.float32)
        in_maps.append({"fb": fbm})
    return in_maps


def get_program_and_maps(x, edge_index, weights):
    global _compiled, _plan_cache
    if _plan_cache is None:
        _plan_cache = _plan(edge_index)
    if _compiled is None:
        _compiled = _build_program(_plan_cache, nqueues=4)
    return _compiled, make_in_maps(x, edge_index, weights, _plan_cache)


def kernel(x, edge_index, Wl1, Wr1, b1, Wl2, Wr2, b2, Wl3, Wr3, b3,
           Wl4, Wr4, b4, _trace=False, _trace_kwargs=None):
    from concourse.bass_utils import run_bass_kernel_spmd

    weights = {
        "Wl1": Wl1, "Wr1": Wr1, "b1": b1,
        "Wl2": Wl2, "Wr2": Wr2, "b2": b2,
        "Wl3": Wl3, "Wr3": Wr3, "b3": b3,
        "Wl4": Wl4, "Wr4": Wr4, "b4": b4,
    }
    nc, in_maps = get_program_and_maps(x, edge_index, weights)
    res = run_bass_kernel_spmd(
        nc, in_maps, core_ids=list(range(NCORES)),
        trace=_trace, **(_trace_kwargs or {}),
    )
    shards = [res.results[c]["out"][:SHARD] for c in range(NCORES)]
    out = np.concatenate(shards, axis=0).astype(np.float32)
    if _trace:
        return out, res
    return out


# revision 5
# speedup vs baseline: 1.1170x; 1.0312x over previous
"""4-layer GraphSAGE (mean aggr) on 8 TRN2 NeuronCores — v4.

Strategy (dst-owner node partitioning, matmul-based segmented aggregation):
  - Nodes partitioned across 8 cores by dst ownership (12500 each, padded
    to 12544 = 98*128).  fp16 node-feature tables (H=128) are replicated
    per core via AllGather each layer; layer 1 is "transform-first":
    phase A computes both table0 = x @ Wl1 (node-major) and the
    feature-major self term y0 = (x @ Wr1)^T, so every layer is uniform.
  - Per layer each core gathers the src rows of its ~200k in-edges with
    dma_gather (int16 idx -> 4 address ranges of 25088 rows).  Edges are
    sorted by (dst-block, src-range) and padded per (block, range) to
    128-edge chunks, with chunk capacities = max over cores so one SPMD
    program serves all 8 cores (pad edges gather row 0, dstrel=-1).
  - Aggregation is a matmul: per 128-edge chunk an on-chip fp16 selection
    matrix S[e, d] = (dstrel[e] == d) * invdeg[e] (2 DVE ops from small
    resident vectors) and aggT_psum[f, d] += gt[e, f].T @ S[e, d].
    PSUM (fp32) accumulates over a (block, range) group; groups drain
    into a resident fp16 SBUF accumulator agg_sb[f, 98, 128].  Mean
    normalization is folded into S via invdeg — no dma_scatter_add, no
    accumulator zeroing/readback.
  - Finalize per block: out_ps[h, d] = Wl.T @ aggT + Wr.T @ hT (self term
    streamed feature-major from DRAM), bias+ReLU on ACT, one PE transpose
    to node-major for the next table / output (fp32 at layer 4).
  - All inputs are packed into ONE fp32 blob per core (per-call dispatch
    cost ~1.6 ms per input buffer + ~0.5 ms/MB of input bytes), with
    fp16/int16 sections accessed via AP bitcast.
  - All PSUM tiles are full 2KB banks: start=True clears has_written for
    the whole bank, so concurrent accumulation groups must not share one.
"""

import numpy as np

# ---------------------------------------------------------------- constants
NCORES = 8
N = 100000
E = 1600000
F_IN = 16
H = 128
SHARD = 12500
BLK = 128
NBLK = 98                # 98*128 = 12544
SHARD_P = NBLK * BLK
TBL_ROWS = NCORES * SHARD_P   # 100352
NRANGE = 4
RANGE_ROWS = TBL_ROWS // NRANGE  # 25088 (< 2**15)
IPG = 8                  # chunks per gather instruction (1024 idxs)
HBLK = 49                # blocks per shard half
HALF_P = HBLK * BLK      # 6272
HTBL = NCORES * HALF_P   # 50176 rows per table half

_compiled = None
_plan_cache = None


# ---------------------------------------------------------------- planning
def _plan(edge_index, ipg=IPG):
    """Shared (cross-core) chunk capacities + per-core slot assignments."""
    src = np.asarray(edge_index[0], dtype=np.int64)
    dst = np.asarray(edge_index[1], dtype=np.int64)
    core = dst // SHARD
    dl = dst - core * SHARD
    blk = dl // BLK
    drel = dl - blk * BLK
    # half-major table rows: row = half*50176 + core*6272 + local, so the
    # AllGather of each shard half fills a contiguous table half (= 2 idx
    # ranges), letting AG(hi) overlap the next layer's range-0/1 gathers
    s_r = src % SHARD
    s_half = s_r // HALF_P
    srow = s_half * (NCORES * HALF_P) + (src // SHARD) * HALF_P \
        + (s_r - s_half * HALF_P)
    rng = srow // RANGE_ROWS
    sloc = srow - rng * RANGE_ROWS

    cnt = np.bincount(dst, minlength=N).astype(np.float32)
    inv_full = (1.0 / np.maximum(cnt, 1.0)).astype(np.float32)
    inv_e = inv_full[dst]

    key = (core * NBLK + blk) * NRANGE + rng
    counts = np.bincount(key, minlength=NCORES * NBLK * NRANGE).reshape(
        NCORES, NBLK, NRANGE)
    cap = -(-counts.max(axis=0) // BLK)          # [NBLK, NRANGE] chunks
    # chunk id layout: half-major, then range, then block — the low half
    # of the shard fully drains at 50% of phase B so AG(lo) launches then
    # and overlaps the entire high-half gather stream
    chunk_base = np.zeros((NBLK, NRANGE), np.int64)
    stream_spans = []                             # (g, start, end)
    off = 0
    for h in range(2):
        bs = range(0, HBLK) if h == 0 else range(HBLK, NBLK)
        for g in range(NRANGE):
            start = off
            for b in bs:
                chunk_base[b, g] = off
                off += int(cap[b, g])
            stream_spans.append((g, start, off))
    QTOT = off
    Qg = cap.sum(axis=0)

    # per-core slot arrays
    per_core = []
    for c in range(NCORES):
        m = core == c
        b_c, g_c, s_c, r_c, iv_c = blk[m], rng[m], sloc[m], drel[m], inv_e[m]
        order = np.lexsort((s_c, g_c, b_c))
        b_c, g_c, s_c, r_c, iv_c = (a[order] for a in (b_c, g_c, s_c, r_c, iv_c))
        k_bg = np.bincount(b_c * NRANGE + g_c, minlength=NBLK * NRANGE
                           ).reshape(NBLK, NRANGE)
        run_start = np.concatenate(
            [[0], np.cumsum(k_bg.reshape(-1))[:-1]]).reshape(NBLK, NRANGE)
        pos_in_run = np.arange(b_c.size) - run_start[b_c, g_c]
        slot = chunk_base[b_c, g_c] * BLK + pos_in_run
        gidx = np.zeros(QTOT * BLK, np.int16)
        drel_s = np.full(QTOT * BLK, -1.0, np.float32)
        inv_s = np.zeros(QTOT * BLK, np.float32)
        gidx[slot] = s_c.astype(np.int16)
        drel_s[slot] = r_c.astype(np.float32)
        inv_s[slot] = iv_c.astype(np.float32)
        per_core.append((gidx, drel_s, inv_s))

    # instruction grouping per (half, range) stream: consecutive IPG chunks
    instrs = []           # (g, q0, k, idx col offset)
    co = 0
    for (g, s, e) in stream_spans:
        q = s
        while q < e:
            k = min(ipg, e - q)
            instrs.append((g, q, k, co))
            co += 8 * k
            q += k
    idxcols = co
    if idxcols % 16:
        idxcols += 16 - idxcols % 16      # keep the packed view fp32-aligned

    return dict(cap=cap, chunk_base=chunk_base, Qg=Qg, QTOT=QTOT,
                instrs=instrs, idxcols=idxcols,
                per_core=per_core, ipg=ipg)


# ---------------------------------------------------------------- blob
def _ceil2(a):
    return a + (a % 2)


def _blob_layout(plan):
    """Column offsets of each section in the packed [128, cols] fp32 blob.

    Sections: xn (node-major x, fp16 [128, NBLK*16]), dr/iv (fp16
    [128, QTOT]), weights (fp16) + biases (fp32), idx (int16, packed
    [128, IDXCOLS/8]).  f16 section sizes are in fp32 columns (=2 fp16).
    """
    QTOT = plan["QTOT"]
    lay = {}
    off = 0
    lay["xn"] = off; off += NBLK * 16 // 2
    lay["dr"] = off; off += _ceil2(QTOT) // 2
    lay["iv"] = off; off += _ceil2(QTOT) // 2
    for l in range(1, 5):
        lay[f"wl{l}"] = off; off += H // 2
        lay[f"wr{l}"] = off; off += H // 2
        lay[f"b{l}"] = off; off += 1
    lay["idx"] = off; off += plan["idxcols"] // 8 // 2
    lay["cols"] = off
    return lay


# ---------------------------------------------------------------- program
def _build_program(plan, no_cc=False, gathers=True, aggs=True, finalize=True,
                   nqueues=1):
    import concourse.bacc as bacc
    import concourse.masks as masks
    import concourse.mybir as mybir
    import concourse.tile as tile

    fp32 = mybir.dt.float32
    fp16 = mybir.dt.float16
    i16 = mybir.dt.int16
    AF = mybir.ActivationFunctionType

    cap = plan["cap"]
    chunk_base = plan["chunk_base"]
    instrs = plan["instrs"]
    QTOT = plan["QTOT"]
    IDXCOLS = plan["idxcols"]
    C8 = IDXCOLS // 8
    ipg = plan["ipg"]

    nonempty = {b: [g for g in range(NRANGE) if cap[b, g] > 0]
                for b in range(NBLK)}
    meta = {}
    for b in range(NBLK):
        for g in range(NRANGE):
            cb, k = int(chunk_base[b, g]), int(cap[b, g])
            for j in range(k):
                meta[cb + j] = (b, g, j == 0, j == k - 1)

    nc = bacc.Bacc(
        "TRN2",
        target_bir_lowering=False,
        debug=False,
        enable_asserts=False,
        num_devices=NCORES,
        num_swdge_queues=nqueues,
    )

    lay = _blob_layout(plan)
    fb_d = nc.dram_tensor("fb", [128, lay["cols"]], fp32, kind="ExternalInput")
    fb = fb_d.ap()
    out_d = nc.dram_tensor("out", [SHARD_P, H], fp32, kind="ExternalOutput")

    def f16sec(off, ncols):
        return fb[:, off:off + _ceil2(ncols) // 2].bitcast(fp16)[:, :ncols]

    with tile.TileContext(nc) as tc:
        with (
            tc.tile_pool(name="dram", bufs=1, space="DRAM") as dpool,
            tc.tile_pool(name="const", bufs=1) as cpool,
        ):
            sh = [[dpool.tile([HALF_P, H], fp16, name=f"sh{l}_{h}")
                   for h in range(2)] for l in range(4)]
            tbls = [[dpool.tile([HTBL, H], fp16,
                                addr_space=("Local" if no_cc else "Shared"),
                                name=f"tbl{l}_{h}") for h in range(2)]
                    for l in range(4)]
            # feature-major self terms: ht[0] = (x @ Wr1)^T, ht[l] = h_l^T
            ht_dram = [dpool.tile([128, SHARD_P], fp16, name=f"ht{l}")
                       for l in range(4)]

            # -------- constants to SBUF
            ident = cpool.tile([128, 128], fp32)
            masks.make_identity(nc, ident[:])
            ident_h = cpool.tile([128, 128], fp16)
            nc.vector.tensor_copy(ident_h[:], ident[:])
            iota3 = cpool.tile([128, ipg, 128], fp16)
            nc.gpsimd.iota(iota3[:], pattern=[[0, ipg], [1, 128]], base=0,
                           channel_multiplier=0,
                           allow_small_or_imprecise_dtypes=True)
            idx_sb = cpool.tile([128, IDXCOLS], i16)
            idx_src = fb[:, lay["idx"]:lay["idx"] + C8 // 2].bitcast(i16)
            idx_src3 = idx_src.rearrange("(a r) c -> r a c", a=8)
            for g in range(8):
                nc.sync.dma_start(
                    idx_sb[16 * g:16 * (g + 1), :].rearrange(
                        "r (a c) -> r a c", a=8),
                    idx_src3)
            dr_sb = cpool.tile([128, QTOT], fp16)
            nc.sync.dma_start(dr_sb[:], f16sec(lay["dr"], QTOT))
            iv_sb = cpool.tile([128, QTOT], fp16)
            nc.sync.dma_start(iv_sb[:], f16sec(lay["iv"], QTOT))
            agg_sb = cpool.tile([128, NBLK, 128], fp16, name="agg")
            xn_sb = cpool.tile([128, NBLK * 16], fp16, name="xn")
            nc.sync.dma_start(xn_sb[:], f16sec(lay["xn"], NBLK * 16))
            xnv = xn_sb.rearrange("p (b f) -> p b f", b=NBLK)
            w_sb = {}
            for l in range(1, 5):
                din = F_IN if l == 1 else H
                for nm in (f"wl{l}", f"wr{l}"):
                    t = cpool.tile([din, H], fp16, name=f"{nm}_sb")
                    nc.sync.dma_start(t[:], f16sec(lay[nm], H)[0:din, :])
                    w_sb[nm] = t
                t = cpool.tile([128, 1], fp32, name=f"b{l}_sb")
                nc.sync.dma_start(t[:], fb[:, lay[f"b{l}"]:lay[f"b{l}"] + 1])
                w_sb[f"b{l}"] = t

            def nm_view(dram_ap):
                return dram_ap.rearrange("(b p) f -> p b f", p=128)

            def allgather(src, dst):
                if no_cc:
                    nc.sync.dma_start(dst[:HALF_P, :], src[:, :])
                    return
                nc.gpsimd.collective_compute(
                    "AllGather",
                    mybir.AluOpType.bypass,
                    replica_groups=[list(range(NCORES))],
                    ins=[src.opt()],
                    outs=[dst.opt()],
                )

            # ---- phase A: table0 = x @ Wl1 (node-major) and
            #               y0^T = (x @ Wr1)^T (feature-major) from
            #               node-major fp16 x resident in SBUF
            with (
                tc.tile_pool(name="phA", bufs=3) as apool,
                tc.tile_pool(name="psA", bufs=2, space="PSUM") as pApool,
                tc.tile_pool(name="psY", bufs=2, space="PSUM") as pYpool,
                tc.tile_pool(name="ptA", bufs=1, space="PSUM") as ptApool,
            ):
                sh0v = [nm_view(s) for s in sh[0]]
                for b0 in range(0, NBLK, 4):
                    nb = min(4, NBLK - b0)
                    cw = nb * 128
                    psA = pApool.tile([128, 512], fp32, tag="psA")
                    psY = pYpool.tile([128, 512], fp32, tag="psY")
                    for j in range(nb):
                        b = b0 + j
                        ptx = ptApool.tile([128, 1024], fp16, tag="ptx")
                        nc.tensor.transpose(ptx[0:16, 0:128], xnv[:, b, :],
                                            ident_h[:])
                        xT = apool.tile([16, 128], fp16, tag="xT")
                        nc.vector.tensor_copy(xT[:], ptx[0:16, 0:128])
                        # disjoint column writes into one bank: only the
                        # first matmul clears has_written
                        nc.tensor.matmul(psA[:, j * 128:(j + 1) * 128],
                                         w_sb["wl1"][:], xT[:],
                                         start=(j == 0), stop=(j == nb - 1))
                        nc.tensor.matmul(psY[:, j * 128:(j + 1) * 128],
                                         w_sb["wr1"][:], xT[:],
                                         start=(j == 0), stop=(j == nb - 1))
                    yt = apool.tile([128, 512], fp16, tag="yt")
                    nc.scalar.copy(yt[:, :cw], psY[:, :cw])
                    nc.sync.dma_start(
                        ht_dram[0][:, b0 * 128:b0 * 128 + cw], yt[:, :cw])
                    tmp = apool.tile([128, 512], fp16, tag="x1tmp")
                    nc.scalar.copy(tmp[:, :cw], psA[:, :cw])
                    ptt = ptApool.tile([128, 1024], fp16, tag="ptt")
                    for j in range(nb):
                        nc.tensor.transpose(
                            ptt[:, j * 128:(j + 1) * 128],
                            tmp[:, j * 128:(j + 1) * 128], ident_h[:])
                    stage = apool.tile([128, 4, 128], fp16, tag="stA")
                    nc.vector.tensor_copy(
                        stage[:, :nb, :],
                        ptt.rearrange("p (a b) -> p a b", a=8)[:, :nb, :])
                    for j in range(nb):
                        b = b0 + j
                        nc.sync.dma_start(
                            sh0v[b // HBLK][:, b % HBLK, :],
                            stage[:, j, :])
                allgather(sh[0][0], tbls[0][0])
                allgather(sh[0][1], tbls[0][1])

            aggv = agg_sb  # [128, NBLK, 128] fp16

            # ---- layers
            with (
                tc.tile_pool(name="gat", bufs=6) as gpool,
                tc.tile_pool(name="sel", bufs=4) as spool,
                tc.tile_pool(name="work", bufs=3) as wpool,
                tc.tile_pool(name="psum_a", bufs=4, space="PSUM") as ppool,
                tc.tile_pool(name="psum_o", bufs=2, space="PSUM") as popool,
                tc.tile_pool(name="psum_t", bufs=1, space="PSUM") as ptpool,
            ):
                for l in range(1, 5):
                    table = tbls[l - 1]
                    for b in range(NBLK):
                        if not nonempty[b]:
                            nc.vector.memset(aggv[:, b, :], 0.0)

                    if l < 4:
                        dst_views = [nm_view(s) for s in sh[l]]
                    else:
                        ov = nm_view(out_d.ap())
                        dst_views = [ov[:, :HBLK, :], ov[:, HBLK:, :]]
                    func = AF.Relu if l < 4 else AF.Identity
                    wl_t = ident_h if l == 1 else w_sb[f"wl{l}"]
                    wr_t = ident_h if l == 1 else w_sb[f"wr{l}"]

                    def emit_finalize(b, l=l, dst_views=dst_views, func=func,
                                      wl_t=wl_t, wr_t=wr_t):
                        hc = wpool.tile([128, 128], fp16, tag="hc")
                        nc.sync.dma_start(
                            hc[:], ht_dram[l - 1][:, b * 128:(b + 1) * 128])
                        ps = popool.tile([128, 512], fp32, tag="po")
                        nc.tensor.matmul(ps[:, :128], wl_t[:], aggv[:, b, :],
                                         start=True, stop=False)
                        nc.tensor.matmul(ps[:, :128], wr_t[:], hc[:],
                                         start=False, stop=True)
                        if l < 4:
                            ev = wpool.tile([128, 128], fp16, tag="ev")
                            nc.scalar.activation(ev[:], ps[:, :128], func,
                                                 bias=w_sb[f"b{l}"][:])
                            nc.sync.dma_start(
                                ht_dram[l][:, b * 128:(b + 1) * 128], ev[:])
                            pt = ptpool.tile([128, 1024], fp16, tag="ptf")
                            nc.tensor.transpose(pt[:, :128], ev[:], ident_h[:])
                            stage = wpool.tile([128, 128], fp16, tag="stg")
                            nc.vector.tensor_copy(stage[:], pt[:, :128])
                            nc.sync.dma_start(
                                dst_views[b // HBLK][:, b % HBLK, :], stage[:])
                        else:
                            ev = wpool.tile([128, 128], fp32, tag="ev4")
                            nc.scalar.activation(ev[:], ps[:, :128], func,
                                                 bias=w_sb[f"b{l}"][:])
                            pt = ptpool.tile([128, 512], fp32, tag="ptf4")
                            nc.tensor.transpose(pt[:, :128], ev[:], ident[:])
                            stage = wpool.tile([128, 128], fp32, tag="stg4")
                            nc.vector.tensor_copy(stage[:], pt[:, :128])
                            nc.sync.dma_start(
                                dst_views[b // HBLK][:, b % HBLK, :], stage[:])

                    # phase B, with finalize interleaved: each block finalizes
                    # as soon as its last range-group drains, so sh_lo is
                    # complete mid-pass and AG(lo) launches early
                    pend_lo = set(range(HBLK))
                    pend_hi = set(range(HBLK, NBLK))
                    if finalize:
                        # blocks with no edges anywhere: agg is memset above
                        for b in range(NBLK):
                            if not nonempty[b]:
                                emit_finalize(b)
                                (pend_lo if b < HBLK else pend_hi).discard(b)
                    open_ps = {}
                    for qi, (g, q0, k, co) in enumerate(instrs):
                        gh = g - 2 * (g // 2)
                        tslice = table[g // 2][
                            gh * RANGE_ROWS:(gh + 1) * RANGE_ROWS, :]
                        gt = gpool.tile([128, ipg, 128], fp16, tag="gt")
                        if gathers:
                            nc.gpsimd.dma_gather(
                                gt[:, :k, :], tslice, idx_sb[:, co:co + 8 * k],
                                num_idxs=128 * k, num_idxs_reg=128 * k,
                                elem_size=H, queue_num=qi % nqueues,
                            )
                        else:
                            nc.vector.memset(gt[:, :k, :], 0.0)
                        if not aggs:
                            continue
                        sm = spool.tile([128, ipg, 128], fp16, tag="sm")
                        nc.vector.tensor_tensor(
                            sm[:, :k, :],
                            dr_sb[:, q0:q0 + k].to_broadcast([128, k, 128]),
                            iota3[:, :k, :],
                            mybir.AluOpType.is_equal)
                        nc.vector.tensor_mul(
                            sm[:, :k, :], sm[:, :k, :],
                            iv_sb[:, q0:q0 + k].to_broadcast([128, k, 128]))
                        for j in range(k):
                            q = q0 + j
                            b, g_, first, last = meta[q]
                            if first:
                                psnew = ppool.tile([128, 512], fp32, tag="pag")
                                open_ps[(b, g_)] = psnew
                            ps = open_ps[(b, g_)]
                            nc.tensor.matmul(ps[:, :128], gt[:, j, :],
                                             sm[:, j, :],
                                             start=first, stop=last)
                            if last:
                                del open_ps[(b, g_)]
                                if g_ == nonempty[b][0]:
                                    nc.vector.tensor_copy(
                                        aggv[:, b, :], ps[:, :128])
                                else:
                                    nc.vector.tensor_add(
                                        aggv[:, b, :], aggv[:, b, :],
                                        ps[:, :128])
                                if finalize and g_ == nonempty[b][-1]:
                                    emit_finalize(b)
                                    pend = pend_lo if b < HBLK else pend_hi
                                    pend.discard(b)
                                    if l < 4 and not pend_lo and b < HBLK:
                                        allgather(sh[l][0], tbls[l][0])
                    if finalize and l < 4 and pend_lo:
                        allgather(sh[l][0], tbls[l][0])
                    if l < 4:
                        allgather(sh[l][1], tbls[l][1])

    nc.compile()
    return nc


# ---------------------------------------------------------------- host side
def make_in_maps(x, edge_index, weights, plan):
    x = np.asarray(x, dtype=np.float32)
    instrs = plan["instrs"]
    QTOT = plan["QTOT"]
    IDXCOLS = plan["idxcols"]
    C8 = IDXCOLS // 8
    ipg = plan["ipg"]
    lay = _blob_layout(plan)

    def put16(fbm, off, arr):
        """fp16 array [rows, cols] -> fp32 view at column offset."""
        a = np.ascontiguousarray(arr, np.float16)
        if a.shape[1] % 2:
            a = np.concatenate(
                [a, np.zeros((a.shape[0], 1), np.float16)], axis=1)
        v = np.ascontiguousarray(a).view(np.float32)
        fbm[0:a.shape[0], off:off + v.shape[1]] = v

    in_maps = []
    for c in range(NCORES):
        gidx, drel_s, inv_s = plan["per_core"][c]
        cols = []
        for (g, q0, k, co) in instrs:
            a = gidx[q0 * BLK:(q0 + k) * BLK]
            cols.append(a.reshape(-1, 16).T)
        idx_w = np.concatenate(cols, axis=1)       # [16, <=IDXCOLS]
        if idx_w.shape[1] < IDXCOLS:
            idx_w = np.concatenate(
                [idx_w, np.zeros((16, IDXCOLS - idx_w.shape[1]), np.int16)],
                axis=1)
        # pack [16, IDXCOLS] -> [128, C8]: row 16a+r = idx_w[r, a*C8:(a+1)*C8]
        pk = np.ascontiguousarray(
            idx_w.reshape(16, 8, C8).transpose(1, 0, 2).reshape(128, C8))

        fbm = np.zeros((128, lay["cols"]), np.float32)
        # node-major x: [12544, 16] -> [98, 128, 16] -> [128, 98*16]
        xs = np.zeros((SHARD_P, F_IN), np.float32)
        xs[:SHARD] = x[c * SHARD:(c + 1) * SHARD]
        xn = xs.reshape(NBLK, 128, F_IN).transpose(1, 0, 2).reshape(128, -1)
        put16(fbm, lay["xn"], xn)
        put16(fbm, lay["dr"], drel_s.reshape(QTOT, BLK).T)
        put16(fbm, lay["iv"], inv_s.reshape(QTOT, BLK).T)
        for l in range(1, 5):
            put16(fbm, lay[f"wl{l}"], np.asarray(weights[f"Wl{l}"]))
            put16(fbm, lay[f"wr{l}"], np.asarray(weights[f"Wr{l}"]))
            fbm[:, lay[f"b{l}"]] = np.asarray(
                weights[f"b{l}"], np.float32).reshape(128)
        fbm[:, lay["idx"]:lay["idx"] + C8 // 2] = pk.view(np.float32)
        in_maps.append({"fb": fbm})
    return in_maps


def get_program_and_maps(x, edge_index, weights):
    global _compiled, _plan_cache
    if _plan_cache is None:
        _plan_cache = _plan(edge_index)
    if _compiled is None:
        _compiled = _build_program(_plan_cache, nqueues=4)
    return _compiled, make_in_maps(x, edge_index, weights, _plan_cache)


def kernel(x, edge_index, Wl1, Wr1, b1, Wl2, Wr2, b2, Wl3, Wr3, b3,
           Wl4, Wr4, b4, _trace=False, _trace_kwargs=None):
    from concourse.bass_utils import run_bass_kernel_spmd

    weights = {
        "Wl1": Wl1, "Wr1": Wr1, "b1": b1,
        "Wl2": Wl2, "Wr2": Wr2, "b2": b2,
        "Wl3": Wl3, "Wr3": Wr3, "b3": b3,
        "Wl4": Wl4, "Wr4": Wr4, "b4": b4,
    }
    nc, in_maps = get_program_and_maps(x, edge_index, weights)
    res = run_bass_kernel_spmd(
        nc, in_maps, core_ids=list(range(NCORES)),
        trace=_trace, **(_trace_kwargs or {}),
    )
    shards = [res.results[c]["out"][:SHARD] for c in range(NCORES)]
    out = np.concatenate(shards, axis=0).astype(np.float32)
    if _trace:
        return out, res
    return out


# revision 6
# speedup vs baseline: 1.2151x; 1.0878x over previous
"""4-layer GraphSAGE (mean aggr) on 8 TRN2 NeuronCores — v4.

Strategy (dst-owner node partitioning, matmul-based segmented aggregation):
  - Nodes partitioned across 8 cores by dst ownership (12500 each, padded
    to 12544 = 98*128).  fp16 node-feature tables (H=128) are replicated
    per core via AllGather each layer; layer 1 is "transform-first":
    phase A computes both table0 = x @ Wl1 (node-major) and the
    feature-major self term y0 = (x @ Wr1)^T, so every layer is uniform.
  - Per layer each core gathers the src rows of its ~200k in-edges with
    dma_gather (int16 idx -> 4 address ranges of 25088 rows).  Edges are
    sorted by (dst-block, src-range) and padded per (block, range) to
    128-edge chunks, with chunk capacities = max over cores so one SPMD
    program serves all 8 cores (pad edges gather row 0, dstrel=-1).
  - Aggregation is a matmul: per 128-edge chunk an on-chip fp16 selection
    matrix S[e, d] = (dstrel[e] == d) * invdeg[e] (2 DVE ops from small
    resident vectors) and aggT_psum[f, d] += gt[e, f].T @ S[e, d].
    PSUM (fp32) accumulates over a (block, range) group; groups drain
    into a resident fp16 SBUF accumulator agg_sb[f, 98, 128].  Mean
    normalization is folded into S via invdeg — no dma_scatter_add, no
    accumulator zeroing/readback.
  - Finalize per block: out_ps[h, d] = Wl.T @ aggT + Wr.T @ hT (self term
    streamed feature-major from DRAM), bias+ReLU on ACT, one PE transpose
    to node-major for the next table / output (fp32 at layer 4).
  - All inputs are packed into ONE fp32 blob per core (per-call dispatch
    cost ~1.6 ms per input buffer + ~0.5 ms/MB of input bytes), with
    fp16/int16 sections accessed via AP bitcast.
  - All PSUM tiles are full 2KB banks: start=True clears has_written for
    the whole bank, so concurrent accumulation groups must not share one.
"""

import numpy as np

# ---------------------------------------------------------------- constants
NCORES = 8
N = 100000
E = 1600000
F_IN = 16
H = 128
SHARD = 12500
BLK = 128
NBLK = 98                # 98*128 = 12544
SHARD_P = NBLK * BLK
TBL_ROWS = NCORES * SHARD_P   # 100352
NRANGE = 4
RANGE_ROWS = TBL_ROWS // NRANGE  # 25088 (< 2**15)
IPG = 8                  # chunks per gather instruction (1024 idxs)
# uneven quarters: integral 128-blocks, each table quarter < 2**15 rows
QB = [25, 25, 24, 24]            # blocks per shard quarter
QBSTART = [0, 25, 50, 74]
QROWS = [q * BLK for q in QB]    # per-core rows: 3200,3200,3072,3072
QRSTART = [0, 3200, 6400, 9472]
QTROWS = [NCORES * r for r in QROWS]
QTSTART = [0, 25600, 51200, 75776]
BQ = [0] * 25 + [1] * 25 + [2] * 24 + [3] * 24   # block -> quarter

_compiled = None
_plan_cache = None


# ---------------------------------------------------------------- planning
def _plan(edge_index, ipg=IPG):
    """Shared (cross-core) chunk capacities + per-core slot assignments."""
    src = np.asarray(edge_index[0], dtype=np.int64)
    dst = np.asarray(edge_index[1], dtype=np.int64)
    core = dst // SHARD
    dl = dst - core * SHARD
    blk = dl // BLK
    drel = dl - blk * BLK
    # quarter-major table rows: range g IS table quarter g, so the
    # AllGather of shard quarter q fills table quarter q and the next
    # layer's range-q gathers depend only on it
    s_r = src % SHARD
    rng = np.searchsorted(np.array(QRSTART[1:]), s_r, side="right")
    sloc = (src // SHARD) * np.array(QROWS)[rng] + \
        (s_r - np.array(QRSTART)[rng])

    cnt = np.bincount(dst, minlength=N).astype(np.float32)
    inv_full = (1.0 / np.maximum(cnt, 1.0)).astype(np.float32)
    inv_e = inv_full[dst]

    key = (core * NBLK + blk) * NRANGE + rng
    counts = np.bincount(key, minlength=NCORES * NBLK * NRANGE).reshape(
        NCORES, NBLK, NRANGE)
    cap = -(-counts.max(axis=0) // BLK)          # [NBLK, NRANGE] chunks
    # chunk id layout: dst-quarter-major, then range, then block — shard
    # quarter q fully drains at (q+1)/4 of phase B so its AG launches
    # there and overlaps the remaining gather streams
    chunk_base = np.zeros((NBLK, NRANGE), np.int64)
    stream_spans = []                             # (g, start, end)
    off = 0
    for qd in range(4):
        bs = range(QBSTART[qd], QBSTART[qd] + QB[qd])
        for g in range(NRANGE):
            start = off
            for b in bs:
                chunk_base[b, g] = off
                off += int(cap[b, g])
            stream_spans.append((g, start, off))
    QTOT = off
    Qg = cap.sum(axis=0)

    # per-core slot arrays
    per_core = []
    for c in range(NCORES):
        m = core == c
        b_c, g_c, s_c, r_c, iv_c = blk[m], rng[m], sloc[m], drel[m], inv_e[m]
        order = np.lexsort((s_c, g_c, b_c))
        b_c, g_c, s_c, r_c, iv_c = (a[order] for a in (b_c, g_c, s_c, r_c, iv_c))
        k_bg = np.bincount(b_c * NRANGE + g_c, minlength=NBLK * NRANGE
                           ).reshape(NBLK, NRANGE)
        run_start = np.concatenate(
            [[0], np.cumsum(k_bg.reshape(-1))[:-1]]).reshape(NBLK, NRANGE)
        pos_in_run = np.arange(b_c.size) - run_start[b_c, g_c]
        slot = chunk_base[b_c, g_c] * BLK + pos_in_run
        gidx = np.zeros(QTOT * BLK, np.int16)
        drel_s = np.full(QTOT * BLK, -1.0, np.float32)
        inv_s = np.zeros(QTOT * BLK, np.float32)
        gidx[slot] = s_c.astype(np.int16)
        drel_s[slot] = r_c.astype(np.float32)
        inv_s[slot] = iv_c.astype(np.float32)
        per_core.append((gidx, drel_s, inv_s))

    # instruction grouping per (half, range) stream: consecutive IPG chunks
    instrs = []           # (g, q0, k, idx col offset)
    co = 0
    for (g, s, e) in stream_spans:
        q = s
        while q < e:
            k = min(ipg, e - q)
            instrs.append((g, q, k, co))
            co += 8 * k
            q += k
    idxcols = co
    if idxcols % 16:
        idxcols += 16 - idxcols % 16      # keep the packed view fp32-aligned

    return dict(cap=cap, chunk_base=chunk_base, Qg=Qg, QTOT=QTOT,
                instrs=instrs, idxcols=idxcols,
                per_core=per_core, ipg=ipg)


# ---------------------------------------------------------------- blob
def _ceil2(a):
    return a + (a % 2)


def _blob_layout(plan):
    """Column offsets of each section in the packed [128, cols] fp32 blob.

    Sections: xn (node-major x, fp16 [128, NBLK*16]), dr/iv (fp16
    [128, QTOT]), weights (fp16) + biases (fp32), idx (int16, packed
    [128, IDXCOLS/8]).  f16 section sizes are in fp32 columns (=2 fp16).
    """
    QTOT = plan["QTOT"]
    lay = {}
    off = 0
    lay["xn"] = off; off += NBLK * 16 // 2
    lay["dr"] = off; off += _ceil2(QTOT) // 2
    lay["iv"] = off; off += _ceil2(QTOT) // 2
    for l in range(1, 5):
        lay[f"wl{l}"] = off; off += H // 2
        lay[f"wr{l}"] = off; off += H // 2
        lay[f"b{l}"] = off; off += 1
    lay["idx"] = off; off += plan["idxcols"] // 8 // 2
    lay["cols"] = off
    return lay


# ---------------------------------------------------------------- program
def _build_program(plan, no_cc=False, gathers=True, aggs=True, finalize=True,
                   nqueues=1):
    import concourse.bacc as bacc
    import concourse.masks as masks
    import concourse.mybir as mybir
    import concourse.tile as tile

    fp32 = mybir.dt.float32
    fp16 = mybir.dt.float16
    i16 = mybir.dt.int16
    AF = mybir.ActivationFunctionType

    cap = plan["cap"]
    chunk_base = plan["chunk_base"]
    instrs = plan["instrs"]
    QTOT = plan["QTOT"]
    IDXCOLS = plan["idxcols"]
    C8 = IDXCOLS // 8
    ipg = plan["ipg"]

    nonempty = {b: [g for g in range(NRANGE) if cap[b, g] > 0]
                for b in range(NBLK)}
    meta = {}
    for b in range(NBLK):
        for g in range(NRANGE):
            cb, k = int(chunk_base[b, g]), int(cap[b, g])
            for j in range(k):
                meta[cb + j] = (b, g, j == 0, j == k - 1)

    nc = bacc.Bacc(
        "TRN2",
        target_bir_lowering=False,
        debug=False,
        enable_asserts=False,
        num_devices=NCORES,
        num_swdge_queues=nqueues,
    )

    lay = _blob_layout(plan)
    fb_d = nc.dram_tensor("fb", [128, lay["cols"]], fp32, kind="ExternalInput")
    fb = fb_d.ap()
    out_d = nc.dram_tensor("out", [SHARD_P, H], fp32, kind="ExternalOutput")

    def f16sec(off, ncols):
        return fb[:, off:off + _ceil2(ncols) // 2].bitcast(fp16)[:, :ncols]

    with tile.TileContext(nc) as tc:
        with (
            tc.tile_pool(name="dram", bufs=1, space="DRAM") as dpool,
            tc.tile_pool(name="const", bufs=1) as cpool,
        ):
            sh = [[dpool.tile([QROWS[h], H], fp16, name=f"sh{l}_{h}")
                   for h in range(4)] for l in range(4)]
            tbls = [[dpool.tile([QTROWS[h], H], fp16,
                                addr_space=("Local" if no_cc else "Shared"),
                                name=f"tbl{l}_{h}") for h in range(4)]
                    for l in range(4)]
            # feature-major self terms: ht[0] = (x @ Wr1)^T, ht[l] = h_l^T
            ht_dram = [dpool.tile([128, SHARD_P], fp16, name=f"ht{l}")
                       for l in range(4)]

            # -------- constants to SBUF
            ident = cpool.tile([128, 128], fp32)
            masks.make_identity(nc, ident[:])
            ident_h = cpool.tile([128, 128], fp16)
            nc.vector.tensor_copy(ident_h[:], ident[:])
            iota3 = cpool.tile([128, ipg, 128], fp16)
            nc.gpsimd.iota(iota3[:], pattern=[[0, ipg], [1, 128]], base=0,
                           channel_multiplier=0,
                           allow_small_or_imprecise_dtypes=True)
            idx_sb = cpool.tile([128, IDXCOLS], i16)
            idx_src = fb[:, lay["idx"]:lay["idx"] + C8 // 2].bitcast(i16)
            idx_src3 = idx_src.rearrange("(a r) c -> r a c", a=8)
            for g in range(8):
                nc.sync.dma_start(
                    idx_sb[16 * g:16 * (g + 1), :].rearrange(
                        "r (a c) -> r a c", a=8),
                    idx_src3)
            dr_sb = cpool.tile([128, QTOT], fp16)
            nc.sync.dma_start(dr_sb[:], f16sec(lay["dr"], QTOT))
            iv_sb = cpool.tile([128, QTOT], fp16)
            nc.sync.dma_start(iv_sb[:], f16sec(lay["iv"], QTOT))
            agg_sb = cpool.tile([128, NBLK, 128], fp16, name="agg")
            xn_sb = cpool.tile([128, NBLK * 16], fp16, name="xn")
            nc.sync.dma_start(xn_sb[:], f16sec(lay["xn"], NBLK * 16))
            xnv = xn_sb.rearrange("p (b f) -> p b f", b=NBLK)
            w_sb = {}
            for l in range(1, 5):
                din = F_IN if l == 1 else H
                for nm in (f"wl{l}", f"wr{l}"):
                    t = cpool.tile([din, H], fp16, name=f"{nm}_sb")
                    nc.sync.dma_start(t[:], f16sec(lay[nm], H)[0:din, :])
                    w_sb[nm] = t
                t = cpool.tile([128, 1], fp32, name=f"b{l}_sb")
                nc.sync.dma_start(t[:], fb[:, lay[f"b{l}"]:lay[f"b{l}"] + 1])
                w_sb[f"b{l}"] = t

            def nm_view(dram_ap):
                return dram_ap.rearrange("(b p) f -> p b f", p=128)

            def allgather(src, dst, q=0):
                if no_cc:
                    nc.sync.dma_start(dst[:QROWS[q], :], src[:, :])
                    return
                nc.gpsimd.collective_compute(
                    "AllGather",
                    mybir.AluOpType.bypass,
                    replica_groups=[list(range(NCORES))],
                    ins=[src.opt()],
                    outs=[dst.opt()],
                )

            # ---- phase A: table0 = x @ Wl1 (node-major) and
            #               y0^T = (x @ Wr1)^T (feature-major) from
            #               node-major fp16 x resident in SBUF
            with (
                tc.tile_pool(name="phA", bufs=3) as apool,
                tc.tile_pool(name="psA", bufs=2, space="PSUM") as pApool,
                tc.tile_pool(name="psY", bufs=2, space="PSUM") as pYpool,
                tc.tile_pool(name="ptA", bufs=1, space="PSUM") as ptApool,
            ):
                sh0v = [nm_view(s) for s in sh[0]]
                pendA = [set(range(QBSTART[q], QBSTART[q] + QB[q]))
                         for q in range(4)]
                for b0 in range(0, NBLK, 4):
                    nb = min(4, NBLK - b0)
                    cw = nb * 128
                    psA = pApool.tile([128, 512], fp32, tag="psA")
                    psY = pYpool.tile([128, 512], fp32, tag="psY")
                    for j in range(nb):
                        b = b0 + j
                        ptx = ptApool.tile([128, 1024], fp16, tag="ptx")
                        nc.tensor.transpose(ptx[0:16, 0:128], xnv[:, b, :],
                                            ident_h[:])
                        xT = apool.tile([16, 128], fp16, tag="xT")
                        nc.vector.tensor_copy(xT[:], ptx[0:16, 0:128])
                        # disjoint column writes into one bank: only the
                        # first matmul clears has_written
                        nc.tensor.matmul(psA[:, j * 128:(j + 1) * 128],
                                         w_sb["wl1"][:], xT[:],
                                         start=(j == 0), stop=(j == nb - 1))
                        nc.tensor.matmul(psY[:, j * 128:(j + 1) * 128],
                                         w_sb["wr1"][:], xT[:],
                                         start=(j == 0), stop=(j == nb - 1))
                    yt = apool.tile([128, 512], fp16, tag="yt")
                    nc.scalar.copy(yt[:, :cw], psY[:, :cw])
                    nc.sync.dma_start(
                        ht_dram[0][:, b0 * 128:b0 * 128 + cw], yt[:, :cw])
                    tmp = apool.tile([128, 512], fp16, tag="x1tmp")
                    nc.scalar.copy(tmp[:, :cw], psA[:, :cw])
                    ptt = ptApool.tile([128, 1024], fp16, tag="ptt")
                    for j in range(nb):
                        nc.tensor.transpose(
                            ptt[:, j * 128:(j + 1) * 128],
                            tmp[:, j * 128:(j + 1) * 128], ident_h[:])
                    stage = apool.tile([128, 4, 128], fp16, tag="stA")
                    nc.vector.tensor_copy(
                        stage[:, :nb, :],
                        ptt.rearrange("p (a b) -> p a b", a=8)[:, :nb, :])
                    for j in range(nb):
                        b = b0 + j
                        q = BQ[b]
                        nc.sync.dma_start(
                            sh0v[q][:, b - QBSTART[q], :], stage[:, j, :])
                        pendA[q].discard(b)
                        if not pendA[q]:
                            pendA[q] = None
                            allgather(sh[0][q], tbls[0][q], q)

            aggv = agg_sb  # [128, NBLK, 128] fp16

            # ---- layers
            with (
                tc.tile_pool(name="gat", bufs=6) as gpool,
                tc.tile_pool(name="sel", bufs=4) as spool,
                tc.tile_pool(name="work", bufs=3) as wpool,
                tc.tile_pool(name="psum_a", bufs=4, space="PSUM") as ppool,
                tc.tile_pool(name="psum_o", bufs=2, space="PSUM") as popool,
                tc.tile_pool(name="psum_t", bufs=1, space="PSUM") as ptpool,
            ):
                for l in range(1, 5):
                    table = tbls[l - 1]
                    for b in range(NBLK):
                        if not nonempty[b]:
                            nc.vector.memset(aggv[:, b, :], 0.0)

                    if l < 4:
                        dst_views = [nm_view(s) for s in sh[l]]
                    else:
                        ov = nm_view(out_d.ap())
                        dst_views = [ov[:, QBSTART[q]:QBSTART[q] + QB[q], :]
                                     for q in range(4)]
                    func = AF.Relu if l < 4 else AF.Identity
                    wl_t = ident_h if l == 1 else w_sb[f"wl{l}"]
                    wr_t = ident_h if l == 1 else w_sb[f"wr{l}"]

                    def emit_finalize(b, l=l, dst_views=dst_views, func=func,
                                      wl_t=wl_t, wr_t=wr_t):
                        hc = wpool.tile([128, 128], fp16, tag="hc")
                        nc.sync.dma_start(
                            hc[:], ht_dram[l - 1][:, b * 128:(b + 1) * 128])
                        ps = popool.tile([128, 512], fp32, tag="po")
                        nc.tensor.matmul(ps[:, :128], wl_t[:], aggv[:, b, :],
                                         start=True, stop=False)
                        nc.tensor.matmul(ps[:, :128], wr_t[:], hc[:],
                                         start=False, stop=True)
                        if l < 4:
                            ev = wpool.tile([128, 128], fp16, tag="ev")
                            nc.scalar.activation(ev[:], ps[:, :128], func,
                                                 bias=w_sb[f"b{l}"][:])
                            nc.sync.dma_start(
                                ht_dram[l][:, b * 128:(b + 1) * 128], ev[:])
                            pt = ptpool.tile([128, 1024], fp16, tag="ptf")
                            nc.tensor.transpose(pt[:, :128], ev[:], ident_h[:])
                            stage = wpool.tile([128, 128], fp16, tag="stg")
                            nc.vector.tensor_copy(stage[:], pt[:, :128])
                            nc.sync.dma_start(
                                dst_views[BQ[b]][:, b - QBSTART[BQ[b]], :],
                                stage[:])
                        else:
                            ev = wpool.tile([128, 128], fp32, tag="ev4")
                            nc.scalar.activation(ev[:], ps[:, :128], func,
                                                 bias=w_sb[f"b{l}"][:])
                            pt = ptpool.tile([128, 512], fp32, tag="ptf4")
                            nc.tensor.transpose(pt[:, :128], ev[:], ident[:])
                            stage = wpool.tile([128, 128], fp32, tag="stg4")
                            nc.vector.tensor_copy(stage[:], pt[:, :128])
                            nc.sync.dma_start(
                                dst_views[BQ[b]][:, b - QBSTART[BQ[b]], :],
                                stage[:])

                    # phase B, with finalize interleaved: each block finalizes
                    # as soon as its last range-group drains, so sh_lo is
                    # complete mid-pass and AG(lo) launches early
                    pend = [set(range(QBSTART[q], QBSTART[q] + QB[q]))
                            for q in range(4)]

                    def block_done(b, l=l, pend=pend):
                        q = BQ[b]
                        pend[q].discard(b)
                        if not pend[q]:
                            pend[q] = {-1}      # fire once
                            if l < 4:
                                allgather(sh[l][q], tbls[l][q], q)

                    if finalize:
                        # blocks with no edges anywhere: agg is memset above
                        for b in range(NBLK):
                            if not nonempty[b]:
                                emit_finalize(b)
                                block_done(b)
                    open_ps = {}
                    for qi, (g, q0, k, co) in enumerate(instrs):
                        tslice = table[g][:, :]
                        gt = gpool.tile([128, ipg, 128], fp16, tag="gt")
                        if gathers:
                            nc.gpsimd.dma_gather(
                                gt[:, :k, :], tslice, idx_sb[:, co:co + 8 * k],
                                num_idxs=128 * k, num_idxs_reg=128 * k,
                                elem_size=H, queue_num=qi % nqueues,
                            )
                        else:
                            nc.vector.memset(gt[:, :k, :], 0.0)
                        if not aggs:
                            continue
                        sm = spool.tile([128, ipg, 128], fp16, tag="sm")
                        nc.vector.tensor_tensor(
                            sm[:, :k, :],
                            dr_sb[:, q0:q0 + k].to_broadcast([128, k, 128]),
                            iota3[:, :k, :],
                            mybir.AluOpType.is_equal)
                        nc.vector.tensor_mul(
                            sm[:, :k, :], sm[:, :k, :],
                            iv_sb[:, q0:q0 + k].to_broadcast([128, k, 128]))
                        for j in range(k):
                            q = q0 + j
                            b, g_, first, last = meta[q]
                            if first:
                                psnew = ppool.tile([128, 512], fp32, tag="pag")
                                open_ps[(b, g_)] = psnew
                            ps = open_ps[(b, g_)]
                            nc.tensor.matmul(ps[:, :128], gt[:, j, :],
                                             sm[:, j, :],
                                             start=first, stop=last)
                            if last:
                                del open_ps[(b, g_)]
                                if g_ == nonempty[b][0]:
                                    nc.vector.tensor_copy(
                                        aggv[:, b, :], ps[:, :128])
                                else:
                                    nc.vector.tensor_add(
                                        aggv[:, b, :], aggv[:, b, :],
                                        ps[:, :128])
                                if finalize and g_ == nonempty[b][-1]:
                                    emit_finalize(b)
                                    block_done(b)

    nc.compile()
    return nc


# ---------------------------------------------------------------- host side
def make_in_maps(x, edge_index, weights, plan):
    x = np.asarray(x, dtype=np.float32)
    instrs = plan["instrs"]
    QTOT = plan["QTOT"]
    IDXCOLS = plan["idxcols"]
    C8 = IDXCOLS // 8
    ipg = plan["ipg"]
    lay = _blob_layout(plan)

    def put16(fbm, off, arr):
        """fp16 array [rows, cols] -> fp32 view at column offset."""
        a = np.ascontiguousarray(arr, np.float16)
        if a.shape[1] % 2:
            a = np.concatenate(
                [a, np.zeros((a.shape[0], 1), np.float16)], axis=1)
        v = np.ascontiguousarray(a).view(np.float32)
        fbm[0:a.shape[0], off:off + v.shape[1]] = v

    in_maps = []
    for c in range(NCORES):
        gidx, drel_s, inv_s = plan["per_core"][c]
        cols = []
        for (g, q0, k, co) in instrs:
            a = gidx[q0 * BLK:(q0 + k) * BLK]
            cols.append(a.reshape(-1, 16).T)
        idx_w = np.concatenate(cols, axis=1)       # [16, <=IDXCOLS]
        if idx_w.shape[1] < IDXCOLS:
            idx_w = np.concatenate(
                [idx_w, np.zeros((16, IDXCOLS - idx_w.shape[1]), np.int16)],
                axis=1)
        # pack [16, IDXCOLS] -> [128, C8]: row 16a+r = idx_w[r, a*C8:(a+1)*C8]
        pk = np.ascontiguousarray(
            idx_w.reshape(16, 8, C8).transpose(1, 0, 2).reshape(128, C8))

        fbm = np.zeros((128, lay["cols"]), np.float32)
        # node-major x: [12544, 16] -> [98, 128, 16] -> [128, 98*16]
        xs = np.zeros((SHARD_P, F_IN), np.float32)
        xs[:SHARD] = x[c * SHARD:(c + 1) * SHARD]
        xn = xs.reshape(NBLK, 128, F_IN).transpose(1, 0, 2).reshape(128, -1)
        put16(fbm, lay["xn"], xn)
        put16(fbm, lay["dr"], drel_s.reshape(QTOT, BLK).T)
        put16(fbm, lay["iv"], inv_s.reshape(QTOT, BLK).T)
        for l in range(1, 5):
            put16(fbm, lay[f"wl{l}"], np.asarray(weights[f"Wl{l}"]))
            put16(fbm, lay[f"wr{l}"], np.asarray(weights[f"Wr{l}"]))
            fbm[:, lay[f"b{l}"]] = np.asarray(
                weights[f"b{l}"], np.float32).reshape(128)
        fbm[:, lay["idx"]:lay["idx"] + C8 // 2] = pk.view(np.float32)
        in_maps.append({"fb": fbm})
    return in_maps


def get_program_and_maps(x, edge_index, weights):
    global _compiled, _plan_cache
    if _plan_cache is None:
        _plan_cache = _plan(edge_index)
    if _compiled is None:
        _compiled = _build_program(_plan_cache, nqueues=4)
    return _compiled, make_in_maps(x, edge_index, weights, _plan_cache)


def kernel(x, edge_index, Wl1, Wr1, b1, Wl2, Wr2, b2, Wl3, Wr3, b3,
           Wl4, Wr4, b4, _trace=False, _trace_kwargs=None):
    from concourse.bass_utils import run_bass_kernel_spmd

    weights = {
        "Wl1": Wl1, "Wr1": Wr1, "b1": b1,
        "Wl2": Wl2, "Wr2": Wr2, "b2": b2,
        "Wl3": Wl3, "Wr3": Wr3, "b3": b3,
        "Wl4": Wl4, "Wr4": Wr4, "b4": b4,
    }
    nc, in_maps = get_program_and_maps(x, edge_index, weights)
    res = run_bass_kernel_spmd(
        nc, in_maps, core_ids=list(range(NCORES)),
        trace=_trace, **(_trace_kwargs or {}),
    )
    shards = [res.results[c]["out"][:SHARD] for c in range(NCORES)]
    out = np.concatenate(shards, axis=0).astype(np.float32)
    if _trace:
        return out, res
    return out
